# revision 22
# baseline (speedup 1.0000x reference)
"""Trainium2 Bass kernel for nn_EquivariantProductBasisBlock.

Architecture (v3):
- Host: sort nodes by species into species-pure windows of WIN nodes
  (NWIN total, WPC per core; data-parallel over nodes on 8 cores).
  For every reduction tile (plane q -> output comp k) the host ships a
  pre-fused stationary matrix  Psi[t][c,f] = lambda_qk[species,c] *
  lin_k[c,f] / sqrt(C)  (bf16), where lambda collapses the CG-structure
  path weights (w1/w2/w3).
- Device, layout [channel=128 partitions, nodes free]:
  * ACT:    squares of the 9 irrep components
  * DVE+GPSIMD: remaining pair products and triple products (batched
    strided tensor_tensor, bf16)
  * PE:     out_k[f,n] += Psi[t]^T @ plane_q[c,n] accumulated in PSUM
            (the species weighting AND channel->feature Linear both live
            in the stationary; one matmul per tile)
- Host: gather windows back to the original node order.
"""

import math
import itertools
import sys

import numpy as np

sys.path.insert(0, "/opt/trn_rl_repo")

N_NODES, CHANNELS, N_ELEM, N_FEAT = 4096, 128, 10, 128
LS_IN = [0, 1, 2]
L_OUT = [0, 1]
L12_MAX = 3
SLICES = {0: (0, 1), 1: (1, 4), 2: (4, 9)}
PATHS1 = [l for l in L_OUT if l in LS_IN]
PATHS2 = [(l1, l2, lo) for l1 in LS_IN for l2 in LS_IN for lo in L_OUT
          if abs(l1 - l2) <= lo <= l1 + l2]
T12_KEYS = [(l1, l2, l12) for l1 in LS_IN for l2 in LS_IN
            for l12 in range(abs(l1 - l2), min(l1 + l2, L12_MAX) + 1)]
PATHS3 = [(k, l3, lo) for k in T12_KEYS for l3 in LS_IN for lo in L_OUT
          if abs(k[2] - l3) <= lo <= k[2] + l3]

WIN = 208          # nodes per window
NWIN = 24          # total windows
WPC = NWIN // 8    # windows per core
PSI_CHT = 33       # psi tiles per DMA chunk


# --------------------------------------------------------------------------
# CG / path-tensor algebra (host-side, numpy)
# --------------------------------------------------------------------------

def _su2_cg(j1, m1, j2, m2, j3, m3):
    if m3 != m1 + m2:
        return 0.0
    f = math.factorial
    vmin = max(-j1 + j2 + m3, -j1 + m1, 0)
    vmax = min(j2 + j3 + m1, j3 - j1 + j2, j3 + m3)
    C = math.sqrt((2 * j3 + 1) * f(j3 + j1 - j2) * f(j3 - j1 + j2)
                  * f(j1 + j2 - j3) * f(j3 + m3) * f(j3 - m3)
                  / (f(j1 + j2 + j3 + 1) * f(j1 - m1) * f(j1 + m1)
                     * f(j2 - m2) * f(j2 + m2)))
    S = 0.0
    for v in range(vmin, vmax + 1):
        S += (-1) ** (v + j2 + m2) * f(j2 + j3 + m1 - v) * f(j1 - m1 + v) / (
            f(v) * f(j3 - j1 + j2 - v) * f(j3 + m3 - v) * f(v + j1 - j2 - m3))
    return C * S


def _c2r(l):
    q = np.zeros((2 * l + 1, 2 * l + 1), dtype=np.complex128)
    for m in range(-l, 0):
        q[l + m, l + abs(m)] = 1.0 / math.sqrt(2)
        q[l + m, l - abs(m)] = -1j / math.sqrt(2)
    q[l, l] = 1.0
    for m in range(1, l + 1):
        q[l + m, l + abs(m)] = (-1) ** m / math.sqrt(2)
        q[l + m, l - abs(m)] = 1j * (-1) ** m / math.sqrt(2)
    return (-1j) ** l * q


_CG_CACHE = {}


def real_cg(l1, l2, l3):
    key = (l1, l2, l3)
    if key not in _CG_CACHE:
        Cc = np.zeros((2 * l1 + 1, 2 * l2 + 1, 2 * l3 + 1), dtype=np.complex128)
        for i, m1 in enumerate(range(-l1, l1 + 1)):
            for j, m2 in enumerate(range(-l2, l2 + 1)):
                for k, m3 in enumerate(range(-l3, l3 + 1)):
                    Cc[i, j, k] = _su2_cg(l1, m1, l2, m2, l3, m3)
        R = np.einsum('ij,kl,mn,ikn->jlm', _c2r(l1), _c2r(l2),
                      np.conj(_c2r(l3).T), Cc)
        _CG_CACHE[key] = np.real(R)
    return _CG_CACHE[key]


def path2_tensor(l1, l2, lo):
    cg = real_cg(l1, l2, lo)
    U = np.zeros((9, 9, 2 * lo + 1))
    s1, e1 = SLICES[l1]
    s2, e2 = SLICES[l2]
    U[s1:e1, s2:e2, :] = cg
    return 0.5 * (U + U.transpose(1, 0, 2))


def path3_tensor(kk, l3, lo):
    l1, l2, l12 = kk
    T = np.einsum('abm,mcn->abcn', real_cg(l1, l2, l12), real_cg(l12, l3, lo))
    U = np.zeros((9, 9, 9, 2 * lo + 1))
    s1, e1 = SLICES[l1]
    s2, e2 = SLICES[l2]
    s3, e3 = SLICES[l3]
    U[s1:e1, s2:e2, s3:e3, :] = T
    S = np.zeros_like(U)
    for perm in itertools.permutations([0, 1, 2]):
        S += U.transpose(*perm, 3)
    return S / 6.0


PAIRS = [(i, j) for i in range(9) for j in range(i, 9)]
PAIR_IDX = {p: n for n, p in enumerate(PAIRS)}
TRIPLES = [(i, j, l) for i in range(9) for j in range(i, 9) for l in range(j, 9)]
TRI_IDX = {t: n for n, t in enumerate(TRIPLES)}


def t2_to_mono(U2):
    v = np.zeros(len(PAIRS))
    for (i, j), n in PAIR_IDX.items():
        v[n] = U2[i, j] * (1 if i == j else 2)
    return v


def t3_to_mono(U3):
    v = np.zeros(len(TRIPLES))
    for (i, j, l), n in TRI_IDX.items():
        v[n] = U3[i, j, l] * len(set(itertools.permutations((i, j, l))))
    return v


def build_functionals():
    F2, F3 = [], []
    for pi, (l1, l2, lo) in enumerate(PATHS2):
        U = path2_tensor(l1, l2, lo)
        if np.abs(U).max() < 1e-12:
            continue
        for m in range(2 * lo + 1):
            k = 0 if lo == 0 else 1 + m
            F2.append((pi, k, t2_to_mono(U[..., m])))
    for pi, (kk, l3, lo) in enumerate(PATHS3):
        U = path3_tensor(kk, l3, lo)
        if np.abs(U).max() < 1e-12:
            continue
        for m in range(2 * lo + 1):
            k = 0 if lo == 0 else 1 + m
            F3.append((pi, k, t3_to_mono(U[..., m])))
    return F2, F3


VV_ORDER = [(1, 1), (2, 2), (3, 3), (1, 2), (1, 3), (2, 3)]
WW_ORDER = ([(i, i) for i in range(4, 9)]
            + [(i, j) for i in range(4, 9) for j in range(i + 1, 9)])
VV_IDX = {p: n for n, p in enumerate(VV_ORDER)}
WW_IDX = {p: n for n, p in enumerate(WW_ORDER)}

# plane layout
#  0..8    A
#  9..14   vv block (diag 9..11 via ACT square, offdiag 12..14)
# 15..29   vw block (i-major)
# 30..44   ww block (diag 30..34 via ACT square, offdiag 35..44)
# 45..53   a0*a_j block (j=0..8)
# 54..83   vv x w          (30)
# 84..128  ww x v          (45)
# 129..146 vv x v          (18)
# 147..?   www exact
# then     a0 triples


def pair_slot_of(i, j):
    if i == 0:
        return 45 + j
    if j <= 3:
        return 9 + VV_IDX[(i, j)]
    if i >= 4:
        return 30 + WW_IDX[(i, j)]
    return 15 + (i - 1) * 5 + (j - 4)


def build_catalog():
    """Returns (prog, nplanes, tiles, coeff).
    prog: list of instr descriptors executed in order:
      ('sq', out0, in0, m)                ACT square block
      ('1d', out0, m, a0, sa, b0, sb)     out[out0+t] = buf[a0+t*sa]*buf[b0+t*sb]
      ('2d', out0, P, i0, L, i1)          out[out0+p*L+l] = buf[i0+p]*buf[i1+l]
    tiles: k-major list of (slot, k); coeff: (deg, path, k) -> [(tile, cf)].
    """
    F2, F3 = build_functionals()

    needed = sorted({TRIPLES[i] for _, _, v in F3
                     for i in np.where(np.abs(v) > 1e-12)[0]})
    www = sorted([t for t in needed if t[0] >= 4],
                 key=lambda t: (pair_slot_of(t[0], t[1]), t[2]))
    www_slot = {}
    www_prog = []
    s = 147
    i = 0
    while i < len(www):
        p0 = pair_slot_of(www[i][0], www[i][1])
        l0 = www[i][2]
        m = 1
        while (i + m < len(www)
               and pair_slot_of(www[i + m][0], www[i + m][1]) == p0
               and www[i + m][2] == l0 + m):
            m += 1
        www_prog.append(('1d', s, m, p0, 0, l0, 1))
        for t in range(m):
            www_slot[www[i + t]] = s + t
        s += m
        i += m
    a0_base = s

    def tri_slot_of(t):
        i, j, l = t
        if i == 0:
            if j == 0:
                return a0_base + 14 + l                     # (0,0,l) l<=3
            if j <= 3 and j == l:
                return a0_base + (j - 1)                    # a0*vv-diag
            if j >= 4 and j == l:
                return a0_base + 3 + (j - 4)                # a0*ww-diag
            return a0_base + 18 + (j - 1) * 5 + (l - 4)     # a0*vw
        if i >= 4:
            return www_slot[t]
        if l <= 3:
            return 129 + VV_IDX[(i, j)] * 3 + (l - 1)       # vv x v
        if j >= 4:
            return 84 + WW_IDX[(j, l)] * 3 + (i - 1)        # ww x v
        return 54 + VV_IDX[(i, j)] * 5 + (l - 4)            # vv x w

    nplanes = a0_base + 18 + 15

    prog = [
        ('sq', 9, 1, 3),                 # vv diag
        ('sq', 30, 4, 5),                # ww diag
        ('1d', 12, 2, 1, 0, 2, 1),       # (1,2),(1,3)
        ('1d', 14, 1, 2, 0, 3, 1),       # (2,3)
        ('2d', 15, 3, 1, 5, 4),          # vw
        ('1d', 35, 4, 4, 0, 5, 1),       # ww offdiag
        ('1d', 39, 3, 5, 0, 6, 1),
        ('1d', 42, 2, 6, 0, 7, 1),
        ('1d', 44, 1, 7, 0, 8, 1),
        ('1d', 45, 9, 0, 0, 0, 1),       # a0 * A
        ('2d', 54, 6, 9, 5, 4),          # vv x w
        ('2d', 84, 15, 30, 3, 1),        # ww x v
        # vv x v exact (9 of 18 used): slots follow 129 + VV_IDX*3 + (l-1)
        ('1d', 129, 3, 9, 0, 1, 1),      # (1,1)x(1,2,3)
        ('1d', 133, 2, 10, 0, 2, 1),     # (2,2)x(2,3)
        ('1d', 137, 1, 11, 0, 3, 1),     # (3,3)x3
        ('1d', 139, 1, 12, 0, 2, 1),     # (1,2)x2
        ('1d', 143, 1, 13, 0, 3, 1),     # (1,3)x3
        ('1d', 146, 1, 14, 0, 3, 1),     # (2,3)x3
    ] + www_prog + [
        ('1d', a0_base, 3, 9, 1, 0, 0),          # a0 * vv-diag
        ('1d', a0_base + 3, 5, 30, 1, 0, 0),     # a0 * ww-diag
        ('1d', a0_base + 18, 15, 15, 1, 0, 0),   # a0 * vw
        ('1d', a0_base + 14, 4, 45, 1, 0, 0),    # a0 * a0a[0:4] -> (0,0,l)
    ]

    tile_set = set()
    for pi in range(len(PATHS1)):
        lo = PATHS1[pi]
        for m in range(2 * lo + 1):
            k = 0 if lo == 0 else 1 + m
            tile_set.add((SLICES[lo][0] + m, k))
    for pi, k, v in F2:
        for i in np.where(np.abs(v) > 1e-12)[0]:
            tile_set.add((pair_slot_of(*PAIRS[i]), k))
    for pi, k, v in F3:
        for i in np.where(np.abs(v) > 1e-12)[0]:
            tile_set.add((tri_slot_of(TRIPLES[i]), k))
    tiles = sorted(tile_set)                               # slot-major
    tidx = {t: n for n, t in enumerate(tiles)}

    coeff = {}
    for pi in range(len(PATHS1)):
        lo = PATHS1[pi]
        for m in range(2 * lo + 1):
            k = 0 if lo == 0 else 1 + m
            coeff.setdefault((1, pi, k), []).append(
                (tidx[(SLICES[lo][0] + m, k)], 1.0))
    for pi, k, v in F2:
        for i in np.where(np.abs(v) > 1e-12)[0]:
            coeff.setdefault((2, pi, k), []).append(
                (tidx[(pair_slot_of(*PAIRS[i]), k)], v[i]))
    for pi, k, v in F3:
        for i in np.where(np.abs(v) > 1e-12)[0]:
            coeff.setdefault((3, pi, k), []).append(
                (tidx[(tri_slot_of(TRIPLES[i]), k)], v[i]))
    return prog, nplanes, tiles, coeff


def build_lambda(tiles, coeff, w1, w2, w3):
    """lam[s, c, tile_index]"""
    wd = {1: w1, 2: w2, 3: w3}
    lam = np.zeros((N_ELEM, CHANNELS, len(tiles)))
    for (d, pi, k), lst in coeff.items():
        w = wd[d][:, pi, :]
        for ti, cf in lst:
            lam[:, :, ti] += w * cf
    return lam


_CATALOG = None


def catalog():
    global _CATALOG
    if _CATALOG is None:
        _CATALOG = build_catalog()
    return _CATALOG


# --------------------------------------------------------------------------
# Bass program
# --------------------------------------------------------------------------

_COMPILED = None


def build_bass(prog, nplanes, tiles):
    from concourse import bacc
    from concourse import tile
    from concourse import mybir

    NT = len(tiles)
    NCH = (NT + PSI_CHT - 1) // PSI_CHT
    bf = mybir.dt.bfloat16
    f32 = mybir.dt.float32
    nc = bacc.Bacc(None, target_bir_lowering=False)

    feats_d = nc.dram_tensor("feats", [WPC, 128, 9, WIN], bf, kind="ExternalInput")
    psi_d = nc.dram_tensor("psi", [WPC, NCH, 128, PSI_CHT, 128], bf,
                           kind="ExternalInput")
    out_d = nc.dram_tensor("out", [WPC, 128, 4, WIN], f32, kind="ExternalOutput")

    # start/stop bookkeeping per k
    first_t = {}
    last_t = {}
    for t, (slot, k) in enumerate(tiles):
        first_t.setdefault(k, t)
        last_t[k] = t

    with tile.TileContext(nc) as tc:
        with (
            tc.tile_pool(name="buf", bufs=2) as bufp,
            tc.tile_pool(name="psir", bufs=3) as psir,
            tc.tile_pool(name="outp", bufs=2) as outp,
            tc.tile_pool(name="psum", bufs=2, space="PSUM") as psump,
        ):
            for w in range(WPC):
                planes = bufp.tile([128, nplanes, WIN], bf, tag="planes")
                nc.sync.dma_start(planes[:, 0:9, :], feats_d[w])

                for ins in prog:
                    if ins[0] == 'sq':
                        _, o, a0, m = ins
                        nc.scalar.activation(
                            planes[:, o:o + m, :], planes[:, a0:a0 + m, :],
                            mybir.ActivationFunctionType.Square)
                    elif ins[0] == '1d':
                        _, o, m, a0, sa, b0, sb = ins
                        if sa == 1:
                            in0 = planes[:, a0:a0 + m, :]
                        else:
                            in0 = planes[:, a0:a0 + 1, :].broadcast_to(
                                [128, m, WIN])
                        if sb == 1:
                            in1 = planes[:, b0:b0 + m, :]
                        else:
                            in1 = planes[:, b0:b0 + 1, :].broadcast_to(
                                [128, m, WIN])
                        nc.vector.tensor_tensor(
                            out=planes[:, o:o + m, :], in0=in0, in1=in1,
                            op=mybir.AluOpType.mult)
                    else:
                        _, o, P, i0, L, i1 = ins
                        out_ap = planes[:, o:o + P * L, :].rearrange(
                            "c (p l) n -> c p l n", p=P)
                        in0 = planes[:, i0:i0 + P, :].unsqueeze(2).broadcast_to(
                            [128, P, L, WIN])
                        in1 = planes[:, i1:i1 + L, :].unsqueeze(1).broadcast_to(
                            [128, P, L, WIN])
                        nc.vector.tensor_tensor(
                            out=out_ap, in0=in0, in1=in1,
                            op=mybir.AluOpType.mult)

                out_ps0 = psump.tile([128, WIN], f32, tag="ops0")
                out_ps1 = psump.tile([128, WIN], f32, tag="ops1")
                out_ps2 = psump.tile([128, WIN], f32, tag="ops2")
                out_ps3 = psump.tile([128, WIN], f32, tag="ops3")
                out_ps = [out_ps0, out_ps1, out_ps2, out_ps3]
                psi_t = None
                for t, (slot, k) in enumerate(tiles):
                    ch, off = divmod(t, PSI_CHT)
                    if off == 0:
                        psi_t = psir.tile([128, PSI_CHT, 128], bf, tag="psi")
                        nc.gpsimd.dma_start(psi_t[:], psi_d[w, ch])
                    nc.tensor.matmul(
                        out_ps[k][:], psi_t[:, off, :], planes[:, slot, :],
                        start=(t == first_t[k]), stop=(t == last_t[k]))

                out_t = outp.tile([128, 4, WIN], f32, tag="out")
                for k in range(4):
                    nc.scalar.activation(
                        out_t[:, k, :], out_ps[k][:],
                        mybir.ActivationFunctionType.Copy)
                nc.gpsimd.dma_start(out_d[w], out_t[:])

    nc.compile()
    return nc


# --------------------------------------------------------------------------
# Host prep + run
# --------------------------------------------------------------------------

def _plan_windows(species):
    order = np.argsort(species, kind='stable')
    win_nodes = np.zeros((NWIN, WIN), np.int64)
    win_spec = np.zeros(NWIN, np.int64)
    win_valid = np.zeros((NWIN, WIN), bool)
    w = 0
    for s in range(N_ELEM):
        idx = order[species[order] == s]
        for c0 in range(0, len(idx), WIN):
            chunk = idx[c0:c0 + WIN]
            n = len(chunk)
            assert w < NWIN, "window overflow"
            win_nodes[w, :n] = chunk
            win_nodes[w, n:] = chunk[0] if n else 0
            win_valid[w, :n] = True
            win_spec[w] = s
            w += 1
    return win_nodes, win_spec, win_valid


_LAST_IN_MAPS = None


def kernel(node_feats, w1, w2, w3, lin_w0, lin_w1, species):
    global _COMPILED, _LAST_IN_MAPS
    import ml_dtypes
    from concourse.bass_utils import run_bass_kernel_spmd

    node_feats = np.ascontiguousarray(np.asarray(node_feats, np.float32))
    species = np.asarray(species)
    prog, nplanes, tiles, coeff = catalog()
    lam = build_lambda(tiles, coeff,
                       np.asarray(w1, np.float64), np.asarray(w2, np.float64),
                       np.asarray(w3, np.float64))       # [S, C, NT] f64
    NT = len(tiles)
    NCH = (NT + PSI_CHT - 1) // PSI_CHT

    if _COMPILED is None:
        _COMPILED = build_bass(prog, nplanes, tiles)
    nc = _COMPILED

    win_nodes, win_spec, win_valid = _plan_windows(species)

    # psi[s, t, c, f] = lam[s, c, t] * lin_{k(t)}[c, f] / sqrt(C)
    sc = 1.0 / math.sqrt(CHANNELS)
    lin = np.stack([np.asarray(lin_w0, np.float64),
                    np.asarray(lin_w1, np.float64)], axis=0) * sc  # [2, C, F]
    ks = np.array([0 if k == 0 else 1 for (slot, k) in tiles])
    lin_per_tile = lin[ks]                                  # [NT, C, F]
    psi_s = np.einsum('sct,tcf->stcf', lam, lin_per_tile)   # [S, NT, C, F]
    psi_s = psi_s.astype(ml_dtypes.bfloat16)

    # per window, chunked layout [NWIN, NCH, C, CHT, F]
    psi_win = np.zeros((NWIN, NCH, CHANNELS, PSI_CHT, N_FEAT), ml_dtypes.bfloat16)
    dummy = ~win_valid.any(axis=1)
    for w in range(NWIN):
        if dummy[w]:
            continue
        p = psi_s[win_spec[w]]                              # [NT, C, F]
        for ch in range(NCH):
            t0 = ch * PSI_CHT
            t1 = min(t0 + PSI_CHT, NT)
            psi_win[w, ch, :, :t1 - t0, :] = p[t0:t1].transpose(1, 0, 2)

    fw = node_feats[win_nodes]                   # [NWIN, WIN, C, 9]
    feats_win = np.ascontiguousarray(
        fw.transpose(0, 2, 3, 1)).astype(ml_dtypes.bfloat16)

    in_maps = []
    for core in range(8):
        ws = slice(core * WPC, (core + 1) * WPC)
        in_maps.append({
            "feats": feats_win[ws],
            "psi": psi_win[ws],
        })
    _LAST_IN_MAPS = in_maps
    res = run_bass_kernel_spmd(nc, in_maps, list(range(8)))

    out = np.zeros((N_NODES, N_FEAT, 4), np.float32)
    for core in range(8):
        o = np.asarray(res.results[core]["out"], np.float32)
        for wi in range(WPC):
            w = core * WPC + wi
            v = win_valid[w]
            if v.any():
                out[win_nodes[w, v]] = o[wi].transpose(2, 0, 1)[v]
    return out


# revision 27
# speedup vs baseline: 1.0417x; 1.0417x over previous
"""Trainium2 Bass kernel for nn_EquivariantProductBasisBlock.

Architecture (v3):
- Host: sort nodes by species into species-pure windows of WIN nodes
  (NWIN total, WPC per core; data-parallel over nodes on 8 cores).
  For every reduction tile (plane q -> output comp k) the host ships a
  pre-fused stationary matrix  Psi[t][c,f] = lambda_qk[species,c] *
  lin_k[c,f] / sqrt(C)  (bf16), where lambda collapses the CG-structure
  path weights (w1/w2/w3).
- Device, layout [channel=128 partitions, nodes free]:
  * ACT:    squares of the 9 irrep components
  * DVE+GPSIMD: remaining pair products and triple products (batched
    strided tensor_tensor, bf16)
  * PE:     out_k[f,n] += Psi[t]^T @ plane_q[c,n] accumulated in PSUM
            (the species weighting AND channel->feature Linear both live
            in the stationary; one matmul per tile)
- Host: gather windows back to the original node order.
"""

import math
import itertools
import sys

import numpy as np

sys.path.insert(0, "/opt/trn_rl_repo")

N_NODES, CHANNELS, N_ELEM, N_FEAT = 4096, 128, 10, 128
LS_IN = [0, 1, 2]
L_OUT = [0, 1]
L12_MAX = 3
SLICES = {0: (0, 1), 1: (1, 4), 2: (4, 9)}
PATHS1 = [l for l in L_OUT if l in LS_IN]
PATHS2 = [(l1, l2, lo) for l1 in LS_IN for l2 in LS_IN for lo in L_OUT
          if abs(l1 - l2) <= lo <= l1 + l2]
T12_KEYS = [(l1, l2, l12) for l1 in LS_IN for l2 in LS_IN
            for l12 in range(abs(l1 - l2), min(l1 + l2, L12_MAX) + 1)]
PATHS3 = [(k, l3, lo) for k in T12_KEYS for l3 in LS_IN for lo in L_OUT
          if abs(k[2] - l3) <= lo <= k[2] + l3]

WIN = 208          # nodes per window
NWIN = 24          # total windows
WPC = NWIN // 8    # windows per core
PSI_CHT = 33       # psi tiles per DMA chunk


# --------------------------------------------------------------------------
# CG / path-tensor algebra (host-side, numpy)
# --------------------------------------------------------------------------

def _su2_cg(j1, m1, j2, m2, j3, m3):
    if m3 != m1 + m2:
        return 0.0
    f = math.factorial
    vmin = max(-j1 + j2 + m3, -j1 + m1, 0)
    vmax = min(j2 + j3 + m1, j3 - j1 + j2, j3 + m3)
    C = math.sqrt((2 * j3 + 1) * f(j3 + j1 - j2) * f(j3 - j1 + j2)
                  * f(j1 + j2 - j3) * f(j3 + m3) * f(j3 - m3)
                  / (f(j1 + j2 + j3 + 1) * f(j1 - m1) * f(j1 + m1)
                     * f(j2 - m2) * f(j2 + m2)))
    S = 0.0
    for v in range(vmin, vmax + 1):
        S += (-1) ** (v + j2 + m2) * f(j2 + j3 + m1 - v) * f(j1 - m1 + v) / (
            f(v) * f(j3 - j1 + j2 - v) * f(j3 + m3 - v) * f(v + j1 - j2 - m3))
    return C * S


def _c2r(l):
    q = np.zeros((2 * l + 1, 2 * l + 1), dtype=np.complex128)
    for m in range(-l, 0):
        q[l + m, l + abs(m)] = 1.0 / math.sqrt(2)
        q[l + m, l - abs(m)] = -1j / math.sqrt(2)
    q[l, l] = 1.0
    for m in range(1, l + 1):
        q[l + m, l + abs(m)] = (-1) ** m / math.sqrt(2)
        q[l + m, l - abs(m)] = 1j * (-1) ** m / math.sqrt(2)
    return (-1j) ** l * q


_CG_CACHE = {}


def real_cg(l1, l2, l3):
    key = (l1, l2, l3)
    if key not in _CG_CACHE:
        Cc = np.zeros((2 * l1 + 1, 2 * l2 + 1, 2 * l3 + 1), dtype=np.complex128)
        for i, m1 in enumerate(range(-l1, l1 + 1)):
            for j, m2 in enumerate(range(-l2, l2 + 1)):
                for k, m3 in enumerate(range(-l3, l3 + 1)):
                    Cc[i, j, k] = _su2_cg(l1, m1, l2, m2, l3, m3)
        R = np.einsum('ij,kl,mn,ikn->jlm', _c2r(l1), _c2r(l2),
                      np.conj(_c2r(l3).T), Cc)
        _CG_CACHE[key] = np.real(R)
    return _CG_CACHE[key]


def path2_tensor(l1, l2, lo):
    cg = real_cg(l1, l2, lo)
    U = np.zeros((9, 9, 2 * lo + 1))
    s1, e1 = SLICES[l1]
    s2, e2 = SLICES[l2]
    U[s1:e1, s2:e2, :] = cg
    return 0.5 * (U + U.transpose(1, 0, 2))


def path3_tensor(kk, l3, lo):
    l1, l2, l12 = kk
    T = np.einsum('abm,mcn->abcn', real_cg(l1, l2, l12), real_cg(l12, l3, lo))
    U = np.zeros((9, 9, 9, 2 * lo + 1))
    s1, e1 = SLICES[l1]
    s2, e2 = SLICES[l2]
    s3, e3 = SLICES[l3]
    U[s1:e1, s2:e2, s3:e3, :] = T
    S = np.zeros_like(U)
    for perm in itertools.permutations([0, 1, 2]):
        S += U.transpose(*perm, 3)
    return S / 6.0


PAIRS = [(i, j) for i in range(9) for j in range(i, 9)]
PAIR_IDX = {p: n for n, p in enumerate(PAIRS)}
TRIPLES = [(i, j, l) for i in range(9) for j in range(i, 9) for l in range(j, 9)]
TRI_IDX = {t: n for n, t in enumerate(TRIPLES)}


def t2_to_mono(U2):
    v = np.zeros(len(PAIRS))
    for (i, j), n in PAIR_IDX.items():
        v[n] = U2[i, j] * (1 if i == j else 2)
    return v


def t3_to_mono(U3):
    v = np.zeros(len(TRIPLES))
    for (i, j, l), n in TRI_IDX.items():
        v[n] = U3[i, j, l] * len(set(itertools.permutations((i, j, l))))
    return v


def build_functionals():
    F2, F3 = [], []
    for pi, (l1, l2, lo) in enumerate(PATHS2):
        U = path2_tensor(l1, l2, lo)
        if np.abs(U).max() < 1e-12:
            continue
        for m in range(2 * lo + 1):
            k = 0 if lo == 0 else 1 + m
            F2.append((pi, k, t2_to_mono(U[..., m])))
    for pi, (kk, l3, lo) in enumerate(PATHS3):
        U = path3_tensor(kk, l3, lo)
        if np.abs(U).max() < 1e-12:
            continue
        for m in range(2 * lo + 1):
            k = 0 if lo == 0 else 1 + m
            F3.append((pi, k, t3_to_mono(U[..., m])))
    return F2, F3


VV_ORDER = [(1, 1), (2, 2), (3, 3), (1, 2), (1, 3), (2, 3)]
WW_ORDER = ([(i, i) for i in range(4, 9)]
            + [(i, j) for i in range(4, 9) for j in range(i + 1, 9)])
VV_IDX = {p: n for n, p in enumerate(VV_ORDER)}
WW_IDX = {p: n for n, p in enumerate(WW_ORDER)}

# plane layout
#  0..8    A
#  9..14   vv block (diag 9..11 via ACT square, offdiag 12..14)
# 15..29   vw block (i-major)
# 30..44   ww block (diag 30..34 via ACT square, offdiag 35..44)
# 45..53   a0*a_j block (j=0..8)
# 54..83   vv x w          (30)
# 84..128  ww x v          (45)
# 129..146 vv x v          (18)
# 147..?   www exact
# then     a0 triples


def pair_slot_of(i, j):
    if i == 0:
        return 45 + j
    if j <= 3:
        return 9 + VV_IDX[(i, j)]
    if i >= 4:
        return 30 + WW_IDX[(i, j)]
    return 15 + (i - 1) * 5 + (j - 4)


def build_catalog():
    """Returns (prog, nplanes, tiles, coeff).
    prog: list of instr descriptors executed in order:
      ('sq', out0, in0, m)                ACT square block
      ('1d', out0, m, a0, sa, b0, sb)     out[out0+t] = buf[a0+t*sa]*buf[b0+t*sb]
      ('2d', out0, P, i0, L, i1)          out[out0+p*L+l] = buf[i0+p]*buf[i1+l]
    tiles: k-major list of (slot, k); coeff: (deg, path, k) -> [(tile, cf)].
    """
    F2, F3 = build_functionals()

    needed = sorted({TRIPLES[i] for _, _, v in F3
                     for i in np.where(np.abs(v) > 1e-12)[0]})
    www = sorted([t for t in needed if t[0] >= 4],
                 key=lambda t: (pair_slot_of(t[0], t[1]), t[2]))
    www_slot = {}
    www_prog = []
    s = 147
    i = 0
    while i < len(www):
        p0 = pair_slot_of(www[i][0], www[i][1])
        l0 = www[i][2]
        m = 1
        while (i + m < len(www)
               and pair_slot_of(www[i + m][0], www[i + m][1]) == p0
               and www[i + m][2] == l0 + m):
            m += 1
        www_prog.append(('1d', s, m, p0, 0, l0, 1))
        for t in range(m):
            www_slot[www[i + t]] = s + t
        s += m
        i += m
    a0_base = s

    def tri_slot_of(t):
        i, j, l = t
        if i == 0:
            if j == 0:
                return a0_base + 14 + l                     # (0,0,l) l<=3
            if j <= 3 and j == l:
                return a0_base + (j - 1)                    # a0*vv-diag
            if j >= 4 and j == l:
                return a0_base + 3 + (j - 4)                # a0*ww-diag
            return a0_base + 18 + (j - 1) * 5 + (l - 4)     # a0*vw
        if i >= 4:
            return www_slot[t]
        if l <= 3:
            return 129 + VV_IDX[(i, j)] * 3 + (l - 1)       # vv x v
        if j >= 4:
            return 84 + WW_IDX[(j, l)] * 3 + (i - 1)        # ww x v
        return 54 + VV_IDX[(i, j)] * 5 + (l - 4)            # vv x w

    nplanes = a0_base + 18 + 15

    # exact instrs for ww-offdiag x v (only used (pair, i) combos; slots
    # follow 84 + WW_IDX*3 + (i-1), unused slots stay garbage/unreferenced)
    wwv_off = []
    used_wwv = {}
    for (i, j, l) in needed:
        if 1 <= i <= 3 and j >= 4 and j != l:
            used_wwv.setdefault(WW_IDX[(j, l)], set()).add(i)
    for idx in sorted(used_wwv):
        vs = sorted(used_wwv[idx])
        r0 = 0
        while r0 < len(vs):
            r1 = r0 + 1
            while r1 < len(vs) and vs[r1] == vs[r1 - 1] + 1:
                r1 += 1
            wwv_off.append(('1d', 84 + idx * 3 + (vs[r0] - 1), r1 - r0,
                            30 + idx, 0, vs[r0], 1))
            r0 = r1

    prog = [
        ('sq', 9, 1, 3),                 # vv diag
        ('sq', 30, 4, 5),                # ww diag
        ('1d', 12, 2, 1, 0, 2, 1),       # (1,2),(1,3)
        ('1d', 14, 1, 2, 0, 3, 1),       # (2,3)
        ('2d', 15, 3, 1, 5, 4),          # vw
        ('1d', 35, 4, 4, 0, 5, 1),       # ww offdiag
        ('1d', 39, 3, 5, 0, 6, 1),
        ('1d', 42, 2, 6, 0, 7, 1),
        ('1d', 44, 1, 7, 0, 8, 1),
        ('1d', 45, 9, 0, 0, 0, 1),       # a0 * A
        ('2d', 54, 6, 9, 5, 4),          # vv x w
        ('2d', 84, 5, 30, 3, 1),         # ww-diag x v (all 15 used)
        # vv x v exact (9 of 18 used): slots follow 129 + VV_IDX*3 + (l-1)
        ('1d', 129, 3, 9, 0, 1, 1),      # (1,1)x(1,2,3)
        ('1d', 133, 2, 10, 0, 2, 1),     # (2,2)x(2,3)
        ('1d', 137, 1, 11, 0, 3, 1),     # (3,3)x3
        ('1d', 139, 1, 12, 0, 2, 1),     # (1,2)x2
        ('1d', 143, 1, 13, 0, 3, 1),     # (1,3)x3
        ('1d', 146, 1, 14, 0, 3, 1),     # (2,3)x3
    ] + wwv_off + www_prog + [
        ('1d', a0_base, 3, 9, 1, 0, 0),          # a0 * vv-diag
        ('1d', a0_base + 3, 5, 30, 1, 0, 0),     # a0 * ww-diag
        ('1d', a0_base + 18, 15, 15, 1, 0, 0),   # a0 * vw
        ('1d', a0_base + 14, 4, 45, 1, 0, 0),    # a0 * a0a[0:4] -> (0,0,l)
    ]

    tile_set = set()
    for pi in range(len(PATHS1)):
        lo = PATHS1[pi]
        for m in range(2 * lo + 1):
            k = 0 if lo == 0 else 1 + m
            tile_set.add((SLICES[lo][0] + m, k))
    for pi, k, v in F2:
        for i in np.where(np.abs(v) > 1e-12)[0]:
            tile_set.add((pair_slot_of(*PAIRS[i]), k))
    for pi, k, v in F3:
        for i in np.where(np.abs(v) > 1e-12)[0]:
            tile_set.add((tri_slot_of(TRIPLES[i]), k))
    tiles = sorted(tile_set)                               # slot-major
    tidx = {t: n for n, t in enumerate(tiles)}

    coeff = {}
    for pi in range(len(PATHS1)):
        lo = PATHS1[pi]
        for m in range(2 * lo + 1):
            k = 0 if lo == 0 else 1 + m
            coeff.setdefault((1, pi, k), []).append(
                (tidx[(SLICES[lo][0] + m, k)], 1.0))
    for pi, k, v in F2:
        for i in np.where(np.abs(v) > 1e-12)[0]:
            coeff.setdefault((2, pi, k), []).append(
                (tidx[(pair_slot_of(*PAIRS[i]), k)], v[i]))
    for pi, k, v in F3:
        for i in np.where(np.abs(v) > 1e-12)[0]:
            coeff.setdefault((3, pi, k), []).append(
                (tidx[(tri_slot_of(TRIPLES[i]), k)], v[i]))
    return prog, nplanes, tiles, coeff


def build_lambda(tiles, coeff, w1, w2, w3):
    """lam[s, c, tile_index]"""
    wd = {1: w1, 2: w2, 3: w3}
    lam = np.zeros((N_ELEM, CHANNELS, len(tiles)))
    for (d, pi, k), lst in coeff.items():
        w = wd[d][:, pi, :]
        for ti, cf in lst:
            lam[:, :, ti] += w * cf
    return lam


_CATALOG = None


def catalog():
    global _CATALOG
    if _CATALOG is None:
        _CATALOG = build_catalog()
    return _CATALOG


# --------------------------------------------------------------------------
# Bass program
# --------------------------------------------------------------------------

_COMPILED = None


def build_bass(prog, nplanes, tiles):
    from concourse import bacc
    from concourse import tile
    from concourse import mybir

    NT = len(tiles)
    NCH = (NT + PSI_CHT - 1) // PSI_CHT
    bf = mybir.dt.bfloat16
    f32 = mybir.dt.float32
    nc = bacc.Bacc(None, target_bir_lowering=False)

    feats_d = nc.dram_tensor("feats", [WPC, 128, 9, WIN], bf, kind="ExternalInput")
    psi_d = nc.dram_tensor("psi", [WPC, NCH, 128, PSI_CHT, 128], bf,
                           kind="ExternalInput")
    out_d = nc.dram_tensor("out", [WPC, 128, 4, WIN], f32, kind="ExternalOutput")

    # start/stop bookkeeping per k
    first_t = {}
    last_t = {}
    for t, (slot, k) in enumerate(tiles):
        first_t.setdefault(k, t)
        last_t[k] = t

    with tile.TileContext(nc) as tc:
        with (
            tc.tile_pool(name="buf", bufs=2) as bufp,
            tc.tile_pool(name="psir", bufs=3) as psir,
            tc.tile_pool(name="outp", bufs=2) as outp,
            tc.tile_pool(name="psum", bufs=2, space="PSUM") as psump,
        ):
            for w in range(WPC):
                planes = bufp.tile([128, nplanes, WIN], bf, tag="planes")
                nc.sync.dma_start(planes[:, 0:9, :], feats_d[w])

                for ins in prog:
                    if ins[0] == 'sq':
                        _, o, a0, m = ins
                        nc.scalar.activation(
                            planes[:, o:o + m, :], planes[:, a0:a0 + m, :],
                            mybir.ActivationFunctionType.Square)
                    elif ins[0] == '1d':
                        _, o, m, a0, sa, b0, sb = ins
                        if sa == 1:
                            in0 = planes[:, a0:a0 + m, :]
                        else:
                            in0 = planes[:, a0:a0 + 1, :].broadcast_to(
                                [128, m, WIN])
                        if sb == 1:
                            in1 = planes[:, b0:b0 + m, :]
                        else:
                            in1 = planes[:, b0:b0 + 1, :].broadcast_to(
                                [128, m, WIN])
                        nc.vector.tensor_tensor(
                            out=planes[:, o:o + m, :], in0=in0, in1=in1,
                            op=mybir.AluOpType.mult)
                    else:
                        _, o, P, i0, L, i1 = ins
                        out_ap = planes[:, o:o + P * L, :].rearrange(
                            "c (p l) n -> c p l n", p=P)
                        in0 = planes[:, i0:i0 + P, :].unsqueeze(2).broadcast_to(
                            [128, P, L, WIN])
                        in1 = planes[:, i1:i1 + L, :].unsqueeze(1).broadcast_to(
                            [128, P, L, WIN])
                        nc.vector.tensor_tensor(
                            out=out_ap, in0=in0, in1=in1,
                            op=mybir.AluOpType.mult)

                out_ps0 = psump.tile([128, WIN], f32, tag="ops0")
                out_ps1 = psump.tile([128, WIN], f32, tag="ops1")
                out_ps2 = psump.tile([128, WIN], f32, tag="ops2")
                out_ps3 = psump.tile([128, WIN], f32, tag="ops3")
                out_ps = [out_ps0, out_ps1, out_ps2, out_ps3]
                psi_t = None
                for t, (slot, k) in enumerate(tiles):
                    ch, off = divmod(t, PSI_CHT)
                    if off == 0:
                        psi_t = psir.tile([128, PSI_CHT, 128], bf, tag="psi")
                        nc.sync.dma_start(psi_t[:], psi_d[w, ch])
                    nc.tensor.matmul(
                        out_ps[k][:], psi_t[:, off, :], planes[:, slot, :],
                        start=(t == first_t[k]), stop=(t == last_t[k]))

                out_t = outp.tile([128, 4, WIN], f32, tag="out")
                for k in range(4):
                    nc.scalar.activation(
                        out_t[:, k, :], out_ps[k][:],
                        mybir.ActivationFunctionType.Copy)
                nc.sync.dma_start(out_d[w], out_t[:])

    nc.compile()
    return nc


# --------------------------------------------------------------------------
# Host prep + run
# --------------------------------------------------------------------------

def _plan_windows(species):
    order = np.argsort(species, kind='stable')
    win_nodes = np.zeros((NWIN, WIN), np.int64)
    win_spec = np.zeros(NWIN, np.int64)
    win_valid = np.zeros((NWIN, WIN), bool)
    w = 0
    for s in range(N_ELEM):
        idx = order[species[order] == s]
        for c0 in range(0, len(idx), WIN):
            chunk = idx[c0:c0 + WIN]
            n = len(chunk)
            assert w < NWIN, "window overflow"
            win_nodes[w, :n] = chunk
            win_nodes[w, n:] = chunk[0] if n else 0
            win_valid[w, :n] = True
            win_spec[w] = s
            w += 1
    return win_nodes, win_spec, win_valid


_LAST_IN_MAPS = None


def kernel(node_feats, w1, w2, w3, lin_w0, lin_w1, species):
    global _COMPILED, _LAST_IN_MAPS
    import ml_dtypes
    from concourse.bass_utils import run_bass_kernel_spmd

    node_feats = np.ascontiguousarray(np.asarray(node_feats, np.float32))
    species = np.asarray(species)
    prog, nplanes, tiles, coeff = catalog()
    lam = build_lambda(tiles, coeff,
                       np.asarray(w1, np.float64), np.asarray(w2, np.float64),
                       np.asarray(w3, np.float64))       # [S, C, NT] f64
    NT = len(tiles)
    NCH = (NT + PSI_CHT - 1) // PSI_CHT

    if _COMPILED is None:
        _COMPILED = build_bass(prog, nplanes, tiles)
    nc = _COMPILED

    win_nodes, win_spec, win_valid = _plan_windows(species)

    # psi[s, t, c, f] = lam[s, c, t] * lin_{k(t)}[c, f] / sqrt(C)
    sc = 1.0 / math.sqrt(CHANNELS)
    lin = np.stack([np.asarray(lin_w0, np.float64),
                    np.asarray(lin_w1, np.float64)], axis=0) * sc  # [2, C, F]
    ks = np.array([0 if k == 0 else 1 for (slot, k) in tiles])
    lin_per_tile = lin[ks]                                  # [NT, C, F]
    psi_s = np.einsum('sct,tcf->stcf', lam, lin_per_tile)   # [S, NT, C, F]
    psi_s = psi_s.astype(ml_dtypes.bfloat16)

    # per window, chunked layout [NWIN, NCH, C, CHT, F]
    psi_win = np.zeros((NWIN, NCH, CHANNELS, PSI_CHT, N_FEAT), ml_dtypes.bfloat16)
    dummy = ~win_valid.any(axis=1)
    for w in range(NWIN):
        if dummy[w]:
            continue
        p = psi_s[win_spec[w]]                              # [NT, C, F]
        for ch in range(NCH):
            t0 = ch * PSI_CHT
            t1 = min(t0 + PSI_CHT, NT)
            psi_win[w, ch, :, :t1 - t0, :] = p[t0:t1].transpose(1, 0, 2)

    fw = node_feats[win_nodes]                   # [NWIN, WIN, C, 9]
    feats_win = np.ascontiguousarray(
        fw.transpose(0, 2, 3, 1)).astype(ml_dtypes.bfloat16)

    in_maps = []
    for core in range(8):
        ws = slice(core * WPC, (core + 1) * WPC)
        in_maps.append({
            "feats": feats_win[ws],
            "psi": psi_win[ws],
        })
    _LAST_IN_MAPS = in_maps
    res = run_bass_kernel_spmd(nc, in_maps, list(range(8)))

    out = np.zeros((N_NODES, N_FEAT, 4), np.float32)
    for core in range(8):
        o = np.asarray(res.results[core]["out"], np.float32)
        for wi in range(WPC):
            w = core * WPC + wi
            v = win_valid[w]
            if v.any():
                out[win_nodes[w, v]] = o[wi].transpose(2, 0, 1)[v]
    return out


# revision 28
# speedup vs baseline: 1.0556x; 1.0134x over previous
"""Trainium2 Bass kernel for nn_EquivariantProductBasisBlock.

Architecture (v3):
- Host: sort nodes by species into species-pure windows of WIN nodes
  (NWIN total, WPC per core; data-parallel over nodes on 8 cores).
  For every reduction tile (plane q -> output comp k) the host ships a
  pre-fused stationary matrix  Psi[t][c,f] = lambda_qk[species,c] *
  lin_k[c,f] / sqrt(C)  (bf16), where lambda collapses the CG-structure
  path weights (w1/w2/w3).
- Device, layout [channel=128 partitions, nodes free]:
  * ACT:    squares of the 9 irrep components
  * DVE+GPSIMD: remaining pair products and triple products (batched
    strided tensor_tensor, bf16)
  * PE:     out_k[f,n] += Psi[t]^T @ plane_q[c,n] accumulated in PSUM
            (the species weighting AND channel->feature Linear both live
            in the stationary; one matmul per tile)
- Host: gather windows back to the original node order.
"""

import math
import itertools
import sys

import numpy as np

sys.path.insert(0, "/opt/trn_rl_repo")

N_NODES, CHANNELS, N_ELEM, N_FEAT = 4096, 128, 10, 128
LS_IN = [0, 1, 2]
L_OUT = [0, 1]
L12_MAX = 3
SLICES = {0: (0, 1), 1: (1, 4), 2: (4, 9)}
PATHS1 = [l for l in L_OUT if l in LS_IN]
PATHS2 = [(l1, l2, lo) for l1 in LS_IN for l2 in LS_IN for lo in L_OUT
          if abs(l1 - l2) <= lo <= l1 + l2]
T12_KEYS = [(l1, l2, l12) for l1 in LS_IN for l2 in LS_IN
            for l12 in range(abs(l1 - l2), min(l1 + l2, L12_MAX) + 1)]
PATHS3 = [(k, l3, lo) for k in T12_KEYS for l3 in LS_IN for lo in L_OUT
          if abs(k[2] - l3) <= lo <= k[2] + l3]

WIN = 208          # nodes per window
NWIN = 24          # total windows
WPC = NWIN // 8    # windows per core
PSI_CHT = 33       # psi tiles per DMA chunk


# --------------------------------------------------------------------------
# CG / path-tensor algebra (host-side, numpy)
# --------------------------------------------------------------------------

def _su2_cg(j1, m1, j2, m2, j3, m3):
    if m3 != m1 + m2:
        return 0.0
    f = math.factorial
    vmin = max(-j1 + j2 + m3, -j1 + m1, 0)
    vmax = min(j2 + j3 + m1, j3 - j1 + j2, j3 + m3)
    C = math.sqrt((2 * j3 + 1) * f(j3 + j1 - j2) * f(j3 - j1 + j2)
                  * f(j1 + j2 - j3) * f(j3 + m3) * f(j3 - m3)
                  / (f(j1 + j2 + j3 + 1) * f(j1 - m1) * f(j1 + m1)
                     * f(j2 - m2) * f(j2 + m2)))
    S = 0.0
    for v in range(vmin, vmax + 1):
        S += (-1) ** (v + j2 + m2) * f(j2 + j3 + m1 - v) * f(j1 - m1 + v) / (
            f(v) * f(j3 - j1 + j2 - v) * f(j3 + m3 - v) * f(v + j1 - j2 - m3))
    return C * S


def _c2r(l):
    q = np.zeros((2 * l + 1, 2 * l + 1), dtype=np.complex128)
    for m in range(-l, 0):
        q[l + m, l + abs(m)] = 1.0 / math.sqrt(2)
        q[l + m, l - abs(m)] = -1j / math.sqrt(2)
    q[l, l] = 1.0
    for m in range(1, l + 1):
        q[l + m, l + abs(m)] = (-1) ** m / math.sqrt(2)
        q[l + m, l - abs(m)] = 1j * (-1) ** m / math.sqrt(2)
    return (-1j) ** l * q


_CG_CACHE = {}


def real_cg(l1, l2, l3):
    key = (l1, l2, l3)
    if key not in _CG_CACHE:
        Cc = np.zeros((2 * l1 + 1, 2 * l2 + 1, 2 * l3 + 1), dtype=np.complex128)
        for i, m1 in enumerate(range(-l1, l1 + 1)):
            for j, m2 in enumerate(range(-l2, l2 + 1)):
                for k, m3 in enumerate(range(-l3, l3 + 1)):
                    Cc[i, j, k] = _su2_cg(l1, m1, l2, m2, l3, m3)
        R = np.einsum('ij,kl,mn,ikn->jlm', _c2r(l1), _c2r(l2),
                      np.conj(_c2r(l3).T), Cc)
        _CG_CACHE[key] = np.real(R)
    return _CG_CACHE[key]


def path2_tensor(l1, l2, lo):
    cg = real_cg(l1, l2, lo)
    U = np.zeros((9, 9, 2 * lo + 1))
    s1, e1 = SLICES[l1]
    s2, e2 = SLICES[l2]
    U[s1:e1, s2:e2, :] = cg
    return 0.5 * (U + U.transpose(1, 0, 2))


def path3_tensor(kk, l3, lo):
    l1, l2, l12 = kk
    T = np.einsum('abm,mcn->abcn', real_cg(l1, l2, l12), real_cg(l12, l3, lo))
    U = np.zeros((9, 9, 9, 2 * lo + 1))
    s1, e1 = SLICES[l1]
    s2, e2 = SLICES[l2]
    s3, e3 = SLICES[l3]
    U[s1:e1, s2:e2, s3:e3, :] = T
    S = np.zeros_like(U)
    for perm in itertools.permutations([0, 1, 2]):
        S += U.transpose(*perm, 3)
    return S / 6.0


PAIRS = [(i, j) for i in range(9) for j in range(i, 9)]
PAIR_IDX = {p: n for n, p in enumerate(PAIRS)}
TRIPLES = [(i, j, l) for i in range(9) for j in range(i, 9) for l in range(j, 9)]
TRI_IDX = {t: n for n, t in enumerate(TRIPLES)}


def t2_to_mono(U2):
    v = np.zeros(len(PAIRS))
    for (i, j), n in PAIR_IDX.items():
        v[n] = U2[i, j] * (1 if i == j else 2)
    return v


def t3_to_mono(U3):
    v = np.zeros(len(TRIPLES))
    for (i, j, l), n in TRI_IDX.items():
        v[n] = U3[i, j, l] * len(set(itertools.permutations((i, j, l))))
    return v


def build_functionals():
    F2, F3 = [], []
    for pi, (l1, l2, lo) in enumerate(PATHS2):
        U = path2_tensor(l1, l2, lo)
        if np.abs(U).max() < 1e-12:
            continue
        for m in range(2 * lo + 1):
            k = 0 if lo == 0 else 1 + m
            F2.append((pi, k, t2_to_mono(U[..., m])))
    for pi, (kk, l3, lo) in enumerate(PATHS3):
        U = path3_tensor(kk, l3, lo)
        if np.abs(U).max() < 1e-12:
            continue
        for m in range(2 * lo + 1):
            k = 0 if lo == 0 else 1 + m
            F3.append((pi, k, t3_to_mono(U[..., m])))
    return F2, F3


VV_ORDER = [(1, 1), (2, 2), (3, 3), (1, 2), (1, 3), (2, 3)]
WW_ORDER = ([(i, i) for i in range(4, 9)]
            + [(i, j) for i in range(4, 9) for j in range(i + 1, 9)])
VV_IDX = {p: n for n, p in enumerate(VV_ORDER)}
WW_IDX = {p: n for n, p in enumerate(WW_ORDER)}

# plane layout
#  0..8    A
#  9..14   vv block (diag 9..11 via ACT square, offdiag 12..14)
# 15..29   vw block (i-major)
# 30..44   ww block (diag 30..34 via ACT square, offdiag 35..44)
# 45..53   a0*a_j block (j=0..8)
# 54..83   vv x w          (30)
# 84..128  ww x v          (45)
# 129..146 vv x v          (18)
# 147..?   www exact
# then     a0 triples


def pair_slot_of(i, j):
    if i == 0:
        return 45 + j
    if j <= 3:
        return 9 + VV_IDX[(i, j)]
    if i >= 4:
        return 30 + WW_IDX[(i, j)]
    return 15 + (i - 1) * 5 + (j - 4)


def build_catalog():
    """Returns (prog, nplanes, tiles, coeff).
    prog: list of instr descriptors executed in order:
      ('sq', out0, in0, m)                ACT square block
      ('1d', out0, m, a0, sa, b0, sb)     out[out0+t] = buf[a0+t*sa]*buf[b0+t*sb]
      ('2d', out0, P, i0, L, i1)          out[out0+p*L+l] = buf[i0+p]*buf[i1+l]
    tiles: k-major list of (slot, k); coeff: (deg, path, k) -> [(tile, cf)].
    """
    F2, F3 = build_functionals()

    needed = sorted({TRIPLES[i] for _, _, v in F3
                     for i in np.where(np.abs(v) > 1e-12)[0]})
    www = sorted([t for t in needed if t[0] >= 4],
                 key=lambda t: (pair_slot_of(t[0], t[1]), t[2]))
    www_slot = {}
    www_prog = []
    s = 147
    i = 0
    while i < len(www):
        p0 = pair_slot_of(www[i][0], www[i][1])
        l0 = www[i][2]
        m = 1
        while (i + m < len(www)
               and pair_slot_of(www[i + m][0], www[i + m][1]) == p0
               and www[i + m][2] == l0 + m):
            m += 1
        www_prog.append(('1d', s, m, p0, 0, l0, 1))
        for t in range(m):
            www_slot[www[i + t]] = s + t
        s += m
        i += m
    a0_base = s

    def tri_slot_of(t):
        i, j, l = t
        if i == 0:
            if j == 0:
                return a0_base + 14 + l                     # (0,0,l) l<=3
            if j <= 3 and j == l:
                return a0_base + (j - 1)                    # a0*vv-diag
            if j >= 4 and j == l:
                return a0_base + 3 + (j - 4)                # a0*ww-diag
            return a0_base + 18 + (j - 1) * 5 + (l - 4)     # a0*vw
        if i >= 4:
            return www_slot[t]
        if l <= 3:
            return 129 + VV_IDX[(i, j)] * 3 + (l - 1)       # vv x v
        if j >= 4:
            return 84 + WW_IDX[(j, l)] * 3 + (i - 1)        # ww x v
        return 54 + VV_IDX[(i, j)] * 5 + (l - 4)            # vv x w

    nplanes = a0_base + 18 + 15

    # exact instrs for ww-offdiag x v (only used (pair, i) combos; slots
    # follow 84 + WW_IDX*3 + (i-1), unused slots stay garbage/unreferenced)
    wwv_off = []
    used_wwv = {}
    for (i, j, l) in needed:
        if 1 <= i <= 3 and j >= 4 and j != l:
            used_wwv.setdefault(WW_IDX[(j, l)], set()).add(i)
    for idx in sorted(used_wwv):
        vs = sorted(used_wwv[idx])
        r0 = 0
        while r0 < len(vs):
            r1 = r0 + 1
            while r1 < len(vs) and vs[r1] == vs[r1 - 1] + 1:
                r1 += 1
            wwv_off.append(('1d', 84 + idx * 3 + (vs[r0] - 1), r1 - r0,
                            30 + idx, 0, vs[r0], 1))
            r0 = r1

    prog = [
        ('sq', 9, 1, 3),                 # vv diag
        ('sq', 30, 4, 5),                # ww diag
        ('1d', 12, 2, 1, 0, 2, 1),       # (1,2),(1,3)
        ('1d', 14, 1, 2, 0, 3, 1),       # (2,3)
        ('2d', 15, 3, 1, 5, 4),          # vw
        ('1d', 35, 4, 4, 0, 5, 1),       # ww offdiag
        ('1d', 39, 3, 5, 0, 6, 1),
        ('1d', 42, 2, 6, 0, 7, 1),
        ('1d', 44, 1, 7, 0, 8, 1),
        ('1d', 45, 4, 0, 0, 0, 1),       # a0 * (a0..a3); (0,j) j>=4 unused
        ('2d', 54, 6, 9, 5, 4),          # vv x w
        ('2d', 84, 5, 30, 3, 1),         # ww-diag x v (all 15 used)
        # vv x v exact (9 of 18 used): slots follow 129 + VV_IDX*3 + (l-1)
        ('1d', 129, 3, 9, 0, 1, 1),      # (1,1)x(1,2,3)
        ('1d', 133, 2, 10, 0, 2, 1),     # (2,2)x(2,3)
        ('1d', 137, 1, 11, 0, 3, 1),     # (3,3)x3
        ('1d', 139, 1, 12, 0, 2, 1),     # (1,2)x2
        ('1d', 143, 1, 13, 0, 3, 1),     # (1,3)x3
        ('1d', 146, 1, 14, 0, 3, 1),     # (2,3)x3
    ] + wwv_off + www_prog + [
        ('1d', a0_base, 3, 9, 1, 0, 0),          # a0 * vv-diag
        ('1d', a0_base + 3, 5, 30, 1, 0, 0),     # a0 * ww-diag
        ('1d', a0_base + 18, 15, 15, 1, 0, 0),   # a0 * vw
        ('1d', a0_base + 14, 4, 45, 1, 0, 0),    # a0 * a0a[0:4] -> (0,0,l)
    ]

    tile_set = set()
    for pi in range(len(PATHS1)):
        lo = PATHS1[pi]
        for m in range(2 * lo + 1):
            k = 0 if lo == 0 else 1 + m
            tile_set.add((SLICES[lo][0] + m, k))
    for pi, k, v in F2:
        for i in np.where(np.abs(v) > 1e-12)[0]:
            tile_set.add((pair_slot_of(*PAIRS[i]), k))
    for pi, k, v in F3:
        for i in np.where(np.abs(v) > 1e-12)[0]:
            tile_set.add((tri_slot_of(TRIPLES[i]), k))
    tiles = sorted(tile_set)                               # slot-major
    tidx = {t: n for n, t in enumerate(tiles)}

    coeff = {}
    for pi in range(len(PATHS1)):
        lo = PATHS1[pi]
        for m in range(2 * lo + 1):
            k = 0 if lo == 0 else 1 + m
            coeff.setdefault((1, pi, k), []).append(
                (tidx[(SLICES[lo][0] + m, k)], 1.0))
    for pi, k, v in F2:
        for i in np.where(np.abs(v) > 1e-12)[0]:
            coeff.setdefault((2, pi, k), []).append(
                (tidx[(pair_slot_of(*PAIRS[i]), k)], v[i]))
    for pi, k, v in F3:
        for i in np.where(np.abs(v) > 1e-12)[0]:
            coeff.setdefault((3, pi, k), []).append(
                (tidx[(tri_slot_of(TRIPLES[i]), k)], v[i]))
    return prog, nplanes, tiles, coeff


def build_lambda(tiles, coeff, w1, w2, w3):
    """lam[s, c, tile_index]"""
    wd = {1: w1, 2: w2, 3: w3}
    lam = np.zeros((N_ELEM, CHANNELS, len(tiles)))
    for (d, pi, k), lst in coeff.items():
        w = wd[d][:, pi, :]
        for ti, cf in lst:
            lam[:, :, ti] += w * cf
    return lam


_CATALOG = None


def catalog():
    global _CATALOG
    if _CATALOG is None:
        _CATALOG = build_catalog()
    return _CATALOG


# --------------------------------------------------------------------------
# Bass program
# --------------------------------------------------------------------------

_COMPILED = None


def build_bass(prog, nplanes, tiles):
    from concourse import bacc
    from concourse import tile
    from concourse import mybir

    NT = len(tiles)
    NCH = (NT + PSI_CHT - 1) // PSI_CHT
    bf = mybir.dt.bfloat16
    f32 = mybir.dt.float32
    nc = bacc.Bacc(None, target_bir_lowering=False)

    feats_d = nc.dram_tensor("feats", [WPC, 128, 9, WIN], bf, kind="ExternalInput")
    psi_d = nc.dram_tensor("psi", [WPC, NCH, 128, PSI_CHT, 128], bf,
                           kind="ExternalInput")
    out_d = nc.dram_tensor("out", [WPC, 128, 4, WIN], f32, kind="ExternalOutput")

    # start/stop bookkeeping per k
    first_t = {}
    last_t = {}
    for t, (slot, k) in enumerate(tiles):
        first_t.setdefault(k, t)
        last_t[k] = t

    with tile.TileContext(nc) as tc:
        with (
            tc.tile_pool(name="buf", bufs=2) as bufp,
            tc.tile_pool(name="psir", bufs=3) as psir,
            tc.tile_pool(name="outp", bufs=2) as outp,
            tc.tile_pool(name="psum", bufs=2, space="PSUM") as psump,
        ):
            for w in range(WPC):
                planes = bufp.tile([128, nplanes, WIN], bf, tag="planes")
                nc.sync.dma_start(planes[:, 0:9, :], feats_d[w])

                for ins in prog:
                    if ins[0] == 'sq':
                        _, o, a0, m = ins
                        nc.scalar.activation(
                            planes[:, o:o + m, :], planes[:, a0:a0 + m, :],
                            mybir.ActivationFunctionType.Square)
                    elif ins[0] == '1d':
                        _, o, m, a0, sa, b0, sb = ins
                        if sa == 1:
                            in0 = planes[:, a0:a0 + m, :]
                        else:
                            in0 = planes[:, a0:a0 + 1, :].broadcast_to(
                                [128, m, WIN])
                        if sb == 1:
                            in1 = planes[:, b0:b0 + m, :]
                        else:
                            in1 = planes[:, b0:b0 + 1, :].broadcast_to(
                                [128, m, WIN])
                        nc.vector.tensor_tensor(
                            out=planes[:, o:o + m, :], in0=in0, in1=in1,
                            op=mybir.AluOpType.mult)
                    else:
                        _, o, P, i0, L, i1 = ins
                        out_ap = planes[:, o:o + P * L, :].rearrange(
                            "c (p l) n -> c p l n", p=P)
                        in0 = planes[:, i0:i0 + P, :].unsqueeze(2).broadcast_to(
                            [128, P, L, WIN])
                        in1 = planes[:, i1:i1 + L, :].unsqueeze(1).broadcast_to(
                            [128, P, L, WIN])
                        nc.vector.tensor_tensor(
                            out=out_ap, in0=in0, in1=in1,
                            op=mybir.AluOpType.mult)

                out_ps0 = psump.tile([128, WIN], f32, tag="ops0")
                out_ps1 = psump.tile([128, WIN], f32, tag="ops1")
                out_ps2 = psump.tile([128, WIN], f32, tag="ops2")
                out_ps3 = psump.tile([128, WIN], f32, tag="ops3")
                out_ps = [out_ps0, out_ps1, out_ps2, out_ps3]
                psi_t = None
                for t, (slot, k) in enumerate(tiles):
                    ch, off = divmod(t, PSI_CHT)
                    if off == 0:
                        psi_t = psir.tile([128, PSI_CHT, 128], bf, tag="psi")
                        nc.sync.dma_start(psi_t[:], psi_d[w, ch])
                    nc.tensor.matmul(
                        out_ps[k][:], psi_t[:, off, :], planes[:, slot, :],
                        start=(t == first_t[k]), stop=(t == last_t[k]))

                out_t = outp.tile([128, 4, WIN], f32, tag="out")
                for k in range(4):
                    nc.scalar.activation(
                        out_t[:, k, :], out_ps[k][:],
                        mybir.ActivationFunctionType.Copy)
                nc.sync.dma_start(out_d[w], out_t[:])

    nc.compile()
    return nc


# --------------------------------------------------------------------------
# Host prep + run
# --------------------------------------------------------------------------

def _plan_windows(species):
    order = np.argsort(species, kind='stable')
    win_nodes = np.zeros((NWIN, WIN), np.int64)
    win_spec = np.zeros(NWIN, np.int64)
    win_valid = np.zeros((NWIN, WIN), bool)
    w = 0
    for s in range(N_ELEM):
        idx = order[species[order] == s]
        for c0 in range(0, len(idx), WIN):
            chunk = idx[c0:c0 + WIN]
            n = len(chunk)
            assert w < NWIN, "window overflow"
            win_nodes[w, :n] = chunk
            win_nodes[w, n:] = chunk[0] if n else 0
            win_valid[w, :n] = True
            win_spec[w] = s
            w += 1
    return win_nodes, win_spec, win_valid


_LAST_IN_MAPS = None


def kernel(node_feats, w1, w2, w3, lin_w0, lin_w1, species):
    global _COMPILED, _LAST_IN_MAPS
    import ml_dtypes
    from concourse.bass_utils import run_bass_kernel_spmd

    node_feats = np.ascontiguousarray(np.asarray(node_feats, np.float32))
    species = np.asarray(species)
    prog, nplanes, tiles, coeff = catalog()
    lam = build_lambda(tiles, coeff,
                       np.asarray(w1, np.float64), np.asarray(w2, np.float64),
                       np.asarray(w3, np.float64))       # [S, C, NT] f64
    NT = len(tiles)
    NCH = (NT + PSI_CHT - 1) // PSI_CHT

    if _COMPILED is None:
        _COMPILED = build_bass(prog, nplanes, tiles)
    nc = _COMPILED

    win_nodes, win_spec, win_valid = _plan_windows(species)

    # psi[s, t, c, f] = lam[s, c, t] * lin_{k(t)}[c, f] / sqrt(C)
    sc = 1.0 / math.sqrt(CHANNELS)
    lin = np.stack([np.asarray(lin_w0, np.float64),
                    np.asarray(lin_w1, np.float64)], axis=0) * sc  # [2, C, F]
    ks = np.array([0 if k == 0 else 1 for (slot, k) in tiles])
    lin_per_tile = lin[ks]                                  # [NT, C, F]
    psi_s = np.einsum('sct,tcf->stcf', lam, lin_per_tile)   # [S, NT, C, F]
    psi_s = psi_s.astype(ml_dtypes.bfloat16)

    # per window, chunked layout [NWIN, NCH, C, CHT, F]
    psi_win = np.zeros((NWIN, NCH, CHANNELS, PSI_CHT, N_FEAT), ml_dtypes.bfloat16)
    dummy = ~win_valid.any(axis=1)
    for w in range(NWIN):
        if dummy[w]:
            continue
        p = psi_s[win_spec[w]]                              # [NT, C, F]
        for ch in range(NCH):
            t0 = ch * PSI_CHT
            t1 = min(t0 + PSI_CHT, NT)
            psi_win[w, ch, :, :t1 - t0, :] = p[t0:t1].transpose(1, 0, 2)

    fw = node_feats[win_nodes]                   # [NWIN, WIN, C, 9]
    feats_win = np.ascontiguousarray(
        fw.transpose(0, 2, 3, 1)).astype(ml_dtypes.bfloat16)

    in_maps = []
    for core in range(8):
        ws = slice(core * WPC, (core + 1) * WPC)
        in_maps.append({
            "feats": feats_win[ws],
            "psi": psi_win[ws],
        })
    _LAST_IN_MAPS = in_maps
    res = run_bass_kernel_spmd(nc, in_maps, list(range(8)))

    out = np.zeros((N_NODES, N_FEAT, 4), np.float32)
    for core in range(8):
        o = np.asarray(res.results[core]["out"], np.float32)
        for wi in range(WPC):
            w = core * WPC + wi
            v = win_valid[w]
            if v.any():
                out[win_nodes[w, v]] = o[wi].transpose(2, 0, 1)[v]
    return out


# revision 29
# speedup vs baseline: 1.0691x; 1.0128x over previous
"""Trainium2 Bass kernel for nn_EquivariantProductBasisBlock.

Architecture (v3):
- Host: sort nodes by species into species-pure windows of WIN nodes
  (NWIN total, WPC per core; data-parallel over nodes on 8 cores).
  For every reduction tile (plane q -> output comp k) the host ships a
  pre-fused stationary matrix  Psi[t][c,f] = lambda_qk[species,c] *
  lin_k[c,f] / sqrt(C)  (bf16), where lambda collapses the CG-structure
  path weights (w1/w2/w3).
- Device, layout [channel=128 partitions, nodes free]:
  * ACT:    squares of the 9 irrep components
  * DVE+GPSIMD: remaining pair products and triple products (batched
    strided tensor_tensor, bf16)
  * PE:     out_k[f,n] += Psi[t]^T @ plane_q[c,n] accumulated in PSUM
            (the species weighting AND channel->feature Linear both live
            in the stationary; one matmul per tile)
- Host: gather windows back to the original node order.
"""

import math
import itertools
import sys

import numpy as np

sys.path.insert(0, "/opt/trn_rl_repo")

N_NODES, CHANNELS, N_ELEM, N_FEAT = 4096, 128, 10, 128
LS_IN = [0, 1, 2]
L_OUT = [0, 1]
L12_MAX = 3
SLICES = {0: (0, 1), 1: (1, 4), 2: (4, 9)}
PATHS1 = [l for l in L_OUT if l in LS_IN]
PATHS2 = [(l1, l2, lo) for l1 in LS_IN for l2 in LS_IN for lo in L_OUT
          if abs(l1 - l2) <= lo <= l1 + l2]
T12_KEYS = [(l1, l2, l12) for l1 in LS_IN for l2 in LS_IN
            for l12 in range(abs(l1 - l2), min(l1 + l2, L12_MAX) + 1)]
PATHS3 = [(k, l3, lo) for k in T12_KEYS for l3 in LS_IN for lo in L_OUT
          if abs(k[2] - l3) <= lo <= k[2] + l3]

WIN = 208          # nodes per window
NWIN = 24          # total windows
WPC = NWIN // 8    # windows per core
PSI_CHT = 33       # psi tiles per DMA chunk


# --------------------------------------------------------------------------
# CG / path-tensor algebra (host-side, numpy)
# --------------------------------------------------------------------------

def _su2_cg(j1, m1, j2, m2, j3, m3):
    if m3 != m1 + m2:
        return 0.0
    f = math.factorial
    vmin = max(-j1 + j2 + m3, -j1 + m1, 0)
    vmax = min(j2 + j3 + m1, j3 - j1 + j2, j3 + m3)
    C = math.sqrt((2 * j3 + 1) * f(j3 + j1 - j2) * f(j3 - j1 + j2)
                  * f(j1 + j2 - j3) * f(j3 + m3) * f(j3 - m3)
                  / (f(j1 + j2 + j3 + 1) * f(j1 - m1) * f(j1 + m1)
                     * f(j2 - m2) * f(j2 + m2)))
    S = 0.0
    for v in range(vmin, vmax + 1):
        S += (-1) ** (v + j2 + m2) * f(j2 + j3 + m1 - v) * f(j1 - m1 + v) / (
            f(v) * f(j3 - j1 + j2 - v) * f(j3 + m3 - v) * f(v + j1 - j2 - m3))
    return C * S


def _c2r(l):
    q = np.zeros((2 * l + 1, 2 * l + 1), dtype=np.complex128)
    for m in range(-l, 0):
        q[l + m, l + abs(m)] = 1.0 / math.sqrt(2)
        q[l + m, l - abs(m)] = -1j / math.sqrt(2)
    q[l, l] = 1.0
    for m in range(1, l + 1):
        q[l + m, l + abs(m)] = (-1) ** m / math.sqrt(2)
        q[l + m, l - abs(m)] = 1j * (-1) ** m / math.sqrt(2)
    return (-1j) ** l * q


_CG_CACHE = {}


def real_cg(l1, l2, l3):
    key = (l1, l2, l3)
    if key not in _CG_CACHE:
        Cc = np.zeros((2 * l1 + 1, 2 * l2 + 1, 2 * l3 + 1), dtype=np.complex128)
        for i, m1 in enumerate(range(-l1, l1 + 1)):
            for j, m2 in enumerate(range(-l2, l2 + 1)):
                for k, m3 in enumerate(range(-l3, l3 + 1)):
                    Cc[i, j, k] = _su2_cg(l1, m1, l2, m2, l3, m3)
        R = np.einsum('ij,kl,mn,ikn->jlm', _c2r(l1), _c2r(l2),
                      np.conj(_c2r(l3).T), Cc)
        _CG_CACHE[key] = np.real(R)
    return _CG_CACHE[key]


def path2_tensor(l1, l2, lo):
    cg = real_cg(l1, l2, lo)
    U = np.zeros((9, 9, 2 * lo + 1))
    s1, e1 = SLICES[l1]
    s2, e2 = SLICES[l2]
    U[s1:e1, s2:e2, :] = cg
    return 0.5 * (U + U.transpose(1, 0, 2))


def path3_tensor(kk, l3, lo):
    l1, l2, l12 = kk
    T = np.einsum('abm,mcn->abcn', real_cg(l1, l2, l12), real_cg(l12, l3, lo))
    U = np.zeros((9, 9, 9, 2 * lo + 1))
    s1, e1 = SLICES[l1]
    s2, e2 = SLICES[l2]
    s3, e3 = SLICES[l3]
    U[s1:e1, s2:e2, s3:e3, :] = T
    S = np.zeros_like(U)
    for perm in itertools.permutations([0, 1, 2]):
        S += U.transpose(*perm, 3)
    return S / 6.0


PAIRS = [(i, j) for i in range(9) for j in range(i, 9)]
PAIR_IDX = {p: n for n, p in enumerate(PAIRS)}
TRIPLES = [(i, j, l) for i in range(9) for j in range(i, 9) for l in range(j, 9)]
TRI_IDX = {t: n for n, t in enumerate(TRIPLES)}


def t2_to_mono(U2):
    v = np.zeros(len(PAIRS))
    for (i, j), n in PAIR_IDX.items():
        v[n] = U2[i, j] * (1 if i == j else 2)
    return v


def t3_to_mono(U3):
    v = np.zeros(len(TRIPLES))
    for (i, j, l), n in TRI_IDX.items():
        v[n] = U3[i, j, l] * len(set(itertools.permutations((i, j, l))))
    return v


def build_functionals():
    F2, F3 = [], []
    for pi, (l1, l2, lo) in enumerate(PATHS2):
        U = path2_tensor(l1, l2, lo)
        if np.abs(U).max() < 1e-12:
            continue
        for m in range(2 * lo + 1):
            k = 0 if lo == 0 else 1 + m
            F2.append((pi, k, t2_to_mono(U[..., m])))
    for pi, (kk, l3, lo) in enumerate(PATHS3):
        U = path3_tensor(kk, l3, lo)
        if np.abs(U).max() < 1e-12:
            continue
        for m in range(2 * lo + 1):
            k = 0 if lo == 0 else 1 + m
            F3.append((pi, k, t3_to_mono(U[..., m])))
    return F2, F3


VV_ORDER = [(1, 1), (2, 2), (3, 3), (1, 2), (1, 3), (2, 3)]
WW_ORDER = ([(i, i) for i in range(4, 9)]
            + [(i, j) for i in range(4, 9) for j in range(i + 1, 9)])
VV_IDX = {p: n for n, p in enumerate(VV_ORDER)}
WW_IDX = {p: n for n, p in enumerate(WW_ORDER)}

# plane layout
#  0..8    A
#  9..14   vv block (diag 9..11 via ACT square, offdiag 12..14)
# 15..29   vw block (i-major)
# 30..44   ww block (diag 30..34 via ACT square, offdiag 35..44)
# 45..53   a0*a_j block (j=0..8)
# 54..83   vv x w          (30)
# 84..128  ww x v          (45)
# 129..146 vv x v          (18)
# 147..?   www exact
# then     a0 triples


def pair_slot_of(i, j):
    if i == 0:
        return 45 + j
    if j <= 3:
        return 9 + VV_IDX[(i, j)]
    if i >= 4:
        return 30 + WW_IDX[(i, j)]
    return 15 + (i - 1) * 5 + (j - 4)


def build_catalog():
    """Returns (prog, nplanes, tiles, coeff).
    prog: list of instr descriptors executed in order:
      ('sq', out0, in0, m)                ACT square block
      ('1d', out0, m, a0, sa, b0, sb)     out[out0+t] = buf[a0+t*sa]*buf[b0+t*sb]
      ('2d', out0, P, i0, L, i1)          out[out0+p*L+l] = buf[i0+p]*buf[i1+l]
    tiles: k-major list of (slot, k); coeff: (deg, path, k) -> [(tile, cf)].
    """
    F2, F3 = build_functionals()

    needed = sorted({TRIPLES[i] for _, _, v in F3
                     for i in np.where(np.abs(v) > 1e-12)[0]})
    www = sorted([t for t in needed if t[0] >= 4],
                 key=lambda t: (pair_slot_of(t[0], t[1]), t[2]))
    www_slot = {}
    www_prog = []
    s = 147
    i = 0
    while i < len(www):
        p0 = pair_slot_of(www[i][0], www[i][1])
        l0 = www[i][2]
        m = 1
        while (i + m < len(www)
               and pair_slot_of(www[i + m][0], www[i + m][1]) == p0
               and www[i + m][2] == l0 + m):
            m += 1
        www_prog.append(('1d', s, m, p0, 0, l0, 1))
        for t in range(m):
            www_slot[www[i + t]] = s + t
        s += m
        i += m
    a0_base = s

    def tri_slot_of(t):
        i, j, l = t
        if i == 0:
            if j == 0:
                return a0_base + 14 + l                     # (0,0,l) l<=3
            if j <= 3 and j == l:
                return a0_base + (j - 1)                    # a0*vv-diag
            if j >= 4 and j == l:
                return a0_base + 3 + (j - 4)                # a0*ww-diag
            return a0_base + 18 + (j - 1) * 5 + (l - 4)     # a0*vw
        if i >= 4:
            return www_slot[t]
        if l <= 3:
            return 129 + VV_IDX[(i, j)] * 3 + (l - 1)       # vv x v
        if j >= 4:
            return 84 + WW_IDX[(j, l)] * 3 + (i - 1)        # ww x v
        return 54 + VV_IDX[(i, j)] * 5 + (l - 4)            # vv x w

    nplanes = a0_base + 18 + 15

    # exact instrs for ww-offdiag x v (only used (pair, i) combos; slots
    # follow 84 + WW_IDX*3 + (i-1), unused slots stay garbage/unreferenced)
    wwv_off = []
    used_wwv = {}
    for (i, j, l) in needed:
        if 1 <= i <= 3 and j >= 4 and j != l:
            used_wwv.setdefault(WW_IDX[(j, l)], set()).add(i)
    for idx in sorted(used_wwv):
        vs = sorted(used_wwv[idx])
        r0 = 0
        while r0 < len(vs):
            r1 = r0 + 1
            while r1 < len(vs) and vs[r1] == vs[r1 - 1] + 1:
                r1 += 1
            wwv_off.append(('1d', 84 + idx * 3 + (vs[r0] - 1), r1 - r0,
                            30 + idx, 0, vs[r0], 1))
            r0 = r1

    prog = [
        ('sq', 9, 1, 3),                 # vv diag
        ('sq', 30, 4, 5),                # ww diag
        ('1d', 12, 2, 1, 0, 2, 1),       # (1,2),(1,3)
        ('1d', 14, 1, 2, 0, 3, 1),       # (2,3)
        ('2d', 15, 3, 1, 5, 4),          # vw
        ('1d', 35, 4, 4, 0, 5, 1),       # ww offdiag
        ('1d', 39, 3, 5, 0, 6, 1),
        ('1d', 42, 2, 6, 0, 7, 1),
        ('1d', 44, 1, 7, 0, 8, 1),
        ('1d', 45, 4, 0, 0, 0, 1),       # a0 * (a0..a3); (0,j) j>=4 unused
        ('2d', 54, 6, 9, 5, 4),          # vv x w
        ('2d', 84, 5, 30, 3, 1),         # ww-diag x v (all 15 used)
        # vv x v exact (9 of 18 used): slots follow 129 + VV_IDX*3 + (l-1)
        ('1d', 129, 3, 9, 0, 1, 1),      # (1,1)x(1,2,3)
        ('1d', 133, 2, 10, 0, 2, 1),     # (2,2)x(2,3)
        ('1d', 137, 1, 11, 0, 3, 1),     # (3,3)x3
        ('1d', 139, 1, 12, 0, 2, 1),     # (1,2)x2
        ('1d', 143, 1, 13, 0, 3, 1),     # (1,3)x3
        ('1d', 146, 1, 14, 0, 3, 1),     # (2,3)x3
    ] + wwv_off + www_prog + [
        ('1d', a0_base, 3, 9, 1, 0, 0),          # a0 * vv-diag
        ('1d', a0_base + 3, 5, 30, 1, 0, 0),     # a0 * ww-diag
        ('1d', a0_base + 18, 15, 15, 1, 0, 0),   # a0 * vw
        ('1d', a0_base + 14, 4, 45, 1, 0, 0),    # a0 * a0a[0:4] -> (0,0,l)
    ]

    tile_set = set()
    for pi in range(len(PATHS1)):
        lo = PATHS1[pi]
        for m in range(2 * lo + 1):
            k = 0 if lo == 0 else 1 + m
            tile_set.add((SLICES[lo][0] + m, k))
    for pi, k, v in F2:
        for i in np.where(np.abs(v) > 1e-12)[0]:
            tile_set.add((pair_slot_of(*PAIRS[i]), k))
    for pi, k, v in F3:
        for i in np.where(np.abs(v) > 1e-12)[0]:
            tile_set.add((tri_slot_of(TRIPLES[i]), k))
    tiles = sorted(tile_set)                               # slot-major
    tidx = {t: n for n, t in enumerate(tiles)}

    coeff = {}
    for pi in range(len(PATHS1)):
        lo = PATHS1[pi]
        for m in range(2 * lo + 1):
            k = 0 if lo == 0 else 1 + m
            coeff.setdefault((1, pi, k), []).append(
                (tidx[(SLICES[lo][0] + m, k)], 1.0))
    for pi, k, v in F2:
        for i in np.where(np.abs(v) > 1e-12)[0]:
            coeff.setdefault((2, pi, k), []).append(
                (tidx[(pair_slot_of(*PAIRS[i]), k)], v[i]))
    for pi, k, v in F3:
        for i in np.where(np.abs(v) > 1e-12)[0]:
            coeff.setdefault((3, pi, k), []).append(
                (tidx[(tri_slot_of(TRIPLES[i]), k)], v[i]))
    return prog, nplanes, tiles, coeff


def build_lambda(tiles, coeff, w1, w2, w3):
    """lam[s, c, tile_index]"""
    wd = {1: w1, 2: w2, 3: w3}
    lam = np.zeros((N_ELEM, CHANNELS, len(tiles)))
    for (d, pi, k), lst in coeff.items():
        w = wd[d][:, pi, :]
        for ti, cf in lst:
            lam[:, :, ti] += w * cf
    return lam


_CATALOG = None


def catalog():
    global _CATALOG
    if _CATALOG is None:
        _CATALOG = build_catalog()
    return _CATALOG


# --------------------------------------------------------------------------
# Bass program
# --------------------------------------------------------------------------

_COMPILED = None


def build_bass(prog, nplanes, tiles):
    from concourse import bacc
    from concourse import tile
    from concourse import mybir

    NT = len(tiles)
    NCH = (NT + PSI_CHT - 1) // PSI_CHT
    bf = mybir.dt.bfloat16
    f32 = mybir.dt.float32
    nc = bacc.Bacc(None, target_bir_lowering=False)

    feats_d = nc.dram_tensor("feats", [WPC, 128, 9, WIN], bf, kind="ExternalInput")
    psi_d = nc.dram_tensor("psi", [WPC, NCH, 128, PSI_CHT, 128], bf,
                           kind="ExternalInput")
    out_d = nc.dram_tensor("out", [WPC, 128, 4, WIN], f32, kind="ExternalOutput")

    # start/stop bookkeeping per k
    first_t = {}
    last_t = {}
    for t, (slot, k) in enumerate(tiles):
        first_t.setdefault(k, t)
        last_t[k] = t

    with tile.TileContext(nc) as tc:
        with (
            tc.tile_pool(name="buf", bufs=2) as bufp,
            tc.tile_pool(name="psir", bufs=4) as psir,
            tc.tile_pool(name="outp", bufs=2) as outp,
            tc.tile_pool(name="psum", bufs=2, space="PSUM") as psump,
        ):
            for w in range(WPC):
                planes = bufp.tile([128, nplanes, WIN], bf, tag="planes")
                nc.sync.dma_start(planes[:, 0:9, :], feats_d[w])

                for ins in prog:
                    if ins[0] == 'sq':
                        _, o, a0, m = ins
                        nc.scalar.activation(
                            planes[:, o:o + m, :], planes[:, a0:a0 + m, :],
                            mybir.ActivationFunctionType.Square)
                    elif ins[0] == '1d':
                        _, o, m, a0, sa, b0, sb = ins
                        if sa == 1:
                            in0 = planes[:, a0:a0 + m, :]
                        else:
                            in0 = planes[:, a0:a0 + 1, :].broadcast_to(
                                [128, m, WIN])
                        if sb == 1:
                            in1 = planes[:, b0:b0 + m, :]
                        else:
                            in1 = planes[:, b0:b0 + 1, :].broadcast_to(
                                [128, m, WIN])
                        nc.vector.tensor_tensor(
                            out=planes[:, o:o + m, :], in0=in0, in1=in1,
                            op=mybir.AluOpType.mult)
                    else:
                        _, o, P, i0, L, i1 = ins
                        out_ap = planes[:, o:o + P * L, :].rearrange(
                            "c (p l) n -> c p l n", p=P)
                        in0 = planes[:, i0:i0 + P, :].unsqueeze(2).broadcast_to(
                            [128, P, L, WIN])
                        in1 = planes[:, i1:i1 + L, :].unsqueeze(1).broadcast_to(
                            [128, P, L, WIN])
                        nc.vector.tensor_tensor(
                            out=out_ap, in0=in0, in1=in1,
                            op=mybir.AluOpType.mult)

                out_ps0 = psump.tile([128, WIN], f32, tag="ops0")
                out_ps1 = psump.tile([128, WIN], f32, tag="ops1")
                out_ps2 = psump.tile([128, WIN], f32, tag="ops2")
                out_ps3 = psump.tile([128, WIN], f32, tag="ops3")
                out_ps = [out_ps0, out_ps1, out_ps2, out_ps3]
                psi_t = None
                for t, (slot, k) in enumerate(tiles):
                    ch, off = divmod(t, PSI_CHT)
                    if off == 0:
                        psi_t = psir.tile([128, PSI_CHT, 128], bf, tag="psi")
                        nc.sync.dma_start(psi_t[:], psi_d[w, ch])
                    nc.tensor.matmul(
                        out_ps[k][:], psi_t[:, off, :], planes[:, slot, :],
                        start=(t == first_t[k]), stop=(t == last_t[k]))

                out_t = outp.tile([128, 4, WIN], f32, tag="out")
                for k in range(4):
                    nc.scalar.activation(
                        out_t[:, k, :], out_ps[k][:],
                        mybir.ActivationFunctionType.Copy)
                nc.sync.dma_start(out_d[w], out_t[:])

    nc.compile()
    return nc


# --------------------------------------------------------------------------
# Host prep + run
# --------------------------------------------------------------------------

def _plan_windows(species):
    order = np.argsort(species, kind='stable')
    win_nodes = np.zeros((NWIN, WIN), np.int64)
    win_spec = np.zeros(NWIN, np.int64)
    win_valid = np.zeros((NWIN, WIN), bool)
    w = 0
    for s in range(N_ELEM):
        idx = order[species[order] == s]
        for c0 in range(0, len(idx), WIN):
            chunk = idx[c0:c0 + WIN]
            n = len(chunk)
            assert w < NWIN, "window overflow"
            win_nodes[w, :n] = chunk
            win_nodes[w, n:] = chunk[0] if n else 0
            win_valid[w, :n] = True
            win_spec[w] = s
            w += 1
    return win_nodes, win_spec, win_valid


_LAST_IN_MAPS = None


def kernel(node_feats, w1, w2, w3, lin_w0, lin_w1, species):
    global _COMPILED, _LAST_IN_MAPS
    import ml_dtypes
    from concourse.bass_utils import run_bass_kernel_spmd

    node_feats = np.ascontiguousarray(np.asarray(node_feats, np.float32))
    species = np.asarray(species)
    prog, nplanes, tiles, coeff = catalog()
    lam = build_lambda(tiles, coeff,
                       np.asarray(w1, np.float64), np.asarray(w2, np.float64),
                       np.asarray(w3, np.float64))       # [S, C, NT] f64
    NT = len(tiles)
    NCH = (NT + PSI_CHT - 1) // PSI_CHT

    if _COMPILED is None:
        _COMPILED = build_bass(prog, nplanes, tiles)
    nc = _COMPILED

    win_nodes, win_spec, win_valid = _plan_windows(species)

    # psi[s, t, c, f] = lam[s, c, t] * lin_{k(t)}[c, f] / sqrt(C)
    sc = 1.0 / math.sqrt(CHANNELS)
    lin = np.stack([np.asarray(lin_w0, np.float64),
                    np.asarray(lin_w1, np.float64)], axis=0) * sc  # [2, C, F]
    ks = np.array([0 if k == 0 else 1 for (slot, k) in tiles])
    lin_per_tile = lin[ks]                                  # [NT, C, F]
    psi_s = np.einsum('sct,tcf->stcf', lam, lin_per_tile)   # [S, NT, C, F]
    psi_s = psi_s.astype(ml_dtypes.bfloat16)

    # per window, chunked layout [NWIN, NCH, C, CHT, F]
    psi_win = np.zeros((NWIN, NCH, CHANNELS, PSI_CHT, N_FEAT), ml_dtypes.bfloat16)
    dummy = ~win_valid.any(axis=1)
    for w in range(NWIN):
        if dummy[w]:
            continue
        p = psi_s[win_spec[w]]                              # [NT, C, F]
        for ch in range(NCH):
            t0 = ch * PSI_CHT
            t1 = min(t0 + PSI_CHT, NT)
            psi_win[w, ch, :, :t1 - t0, :] = p[t0:t1].transpose(1, 0, 2)

    fw = node_feats[win_nodes]                   # [NWIN, WIN, C, 9]
    feats_win = np.ascontiguousarray(
        fw.transpose(0, 2, 3, 1)).astype(ml_dtypes.bfloat16)

    in_maps = []
    for core in range(8):
        ws = slice(core * WPC, (core + 1) * WPC)
        in_maps.append({
            "feats": feats_win[ws],
            "psi": psi_win[ws],
        })
    _LAST_IN_MAPS = in_maps
    res = run_bass_kernel_spmd(nc, in_maps, list(range(8)))

    out = np.zeros((N_NODES, N_FEAT, 4), np.float32)
    for core in range(8):
        o = np.asarray(res.results[core]["out"], np.float32)
        for wi in range(WPC):
            w = core * WPC + wi
            v = win_valid[w]
            if v.any():
                out[win_nodes[w, v]] = o[wi].transpose(2, 0, 1)[v]
    return out


# revision 32
# speedup vs baseline: 1.0822x; 1.0122x over previous
"""Trainium2 Bass kernel for nn_EquivariantProductBasisBlock.

Architecture (v3):
- Host: sort nodes by species into species-pure windows of WIN nodes
  (NWIN total, WPC per core; data-parallel over nodes on 8 cores).
  For every reduction tile (plane q -> output comp k) the host ships a
  pre-fused stationary matrix  Psi[t][c,f] = lambda_qk[species,c] *
  lin_k[c,f] / sqrt(C)  (bf16), where lambda collapses the CG-structure
  path weights (w1/w2/w3).
- Device, layout [channel=128 partitions, nodes free]:
  * ACT:    squares of the 9 irrep components
  * DVE+GPSIMD: remaining pair products and triple products (batched
    strided tensor_tensor, bf16)
  * PE:     out_k[f,n] += Psi[t]^T @ plane_q[c,n] accumulated in PSUM
            (the species weighting AND channel->feature Linear both live
            in the stationary; one matmul per tile)
- Host: gather windows back to the original node order.
"""

import math
import itertools
import sys

import numpy as np

sys.path.insert(0, "/opt/trn_rl_repo")

N_NODES, CHANNELS, N_ELEM, N_FEAT = 4096, 128, 10, 128
LS_IN = [0, 1, 2]
L_OUT = [0, 1]
L12_MAX = 3
SLICES = {0: (0, 1), 1: (1, 4), 2: (4, 9)}
PATHS1 = [l for l in L_OUT if l in LS_IN]
PATHS2 = [(l1, l2, lo) for l1 in LS_IN for l2 in LS_IN for lo in L_OUT
          if abs(l1 - l2) <= lo <= l1 + l2]
T12_KEYS = [(l1, l2, l12) for l1 in LS_IN for l2 in LS_IN
            for l12 in range(abs(l1 - l2), min(l1 + l2, L12_MAX) + 1)]
PATHS3 = [(k, l3, lo) for k in T12_KEYS for l3 in LS_IN for lo in L_OUT
          if abs(k[2] - l3) <= lo <= k[2] + l3]

WIN = 208          # nodes per window
NWIN = 24          # total windows
WPC = NWIN // 8    # windows per core
PSI_CHT = 33       # psi tiles per DMA chunk


# --------------------------------------------------------------------------
# CG / path-tensor algebra (host-side, numpy)
# --------------------------------------------------------------------------

def _su2_cg(j1, m1, j2, m2, j3, m3):
    if m3 != m1 + m2:
        return 0.0
    f = math.factorial
    vmin = max(-j1 + j2 + m3, -j1 + m1, 0)
    vmax = min(j2 + j3 + m1, j3 - j1 + j2, j3 + m3)
    C = math.sqrt((2 * j3 + 1) * f(j3 + j1 - j2) * f(j3 - j1 + j2)
                  * f(j1 + j2 - j3) * f(j3 + m3) * f(j3 - m3)
                  / (f(j1 + j2 + j3 + 1) * f(j1 - m1) * f(j1 + m1)
                     * f(j2 - m2) * f(j2 + m2)))
    S = 0.0
    for v in range(vmin, vmax + 1):
        S += (-1) ** (v + j2 + m2) * f(j2 + j3 + m1 - v) * f(j1 - m1 + v) / (
            f(v) * f(j3 - j1 + j2 - v) * f(j3 + m3 - v) * f(v + j1 - j2 - m3))
    return C * S


def _c2r(l):
    q = np.zeros((2 * l + 1, 2 * l + 1), dtype=np.complex128)
    for m in range(-l, 0):
        q[l + m, l + abs(m)] = 1.0 / math.sqrt(2)
        q[l + m, l - abs(m)] = -1j / math.sqrt(2)
    q[l, l] = 1.0
    for m in range(1, l + 1):
        q[l + m, l + abs(m)] = (-1) ** m / math.sqrt(2)
        q[l + m, l - abs(m)] = 1j * (-1) ** m / math.sqrt(2)
    return (-1j) ** l * q


_CG_CACHE = {}


def real_cg(l1, l2, l3):
    key = (l1, l2, l3)
    if key not in _CG_CACHE:
        Cc = np.zeros((2 * l1 + 1, 2 * l2 + 1, 2 * l3 + 1), dtype=np.complex128)
        for i, m1 in enumerate(range(-l1, l1 + 1)):
            for j, m2 in enumerate(range(-l2, l2 + 1)):
                for k, m3 in enumerate(range(-l3, l3 + 1)):
                    Cc[i, j, k] = _su2_cg(l1, m1, l2, m2, l3, m3)
        R = np.einsum('ij,kl,mn,ikn->jlm', _c2r(l1), _c2r(l2),
                      np.conj(_c2r(l3).T), Cc)
        _CG_CACHE[key] = np.real(R)
    return _CG_CACHE[key]


def path2_tensor(l1, l2, lo):
    cg = real_cg(l1, l2, lo)
    U = np.zeros((9, 9, 2 * lo + 1))
    s1, e1 = SLICES[l1]
    s2, e2 = SLICES[l2]
    U[s1:e1, s2:e2, :] = cg
    return 0.5 * (U + U.transpose(1, 0, 2))


def path3_tensor(kk, l3, lo):
    l1, l2, l12 = kk
    T = np.einsum('abm,mcn->abcn', real_cg(l1, l2, l12), real_cg(l12, l3, lo))
    U = np.zeros((9, 9, 9, 2 * lo + 1))
    s1, e1 = SLICES[l1]
    s2, e2 = SLICES[l2]
    s3, e3 = SLICES[l3]
    U[s1:e1, s2:e2, s3:e3, :] = T
    S = np.zeros_like(U)
    for perm in itertools.permutations([0, 1, 2]):
        S += U.transpose(*perm, 3)
    return S / 6.0


PAIRS = [(i, j) for i in range(9) for j in range(i, 9)]
PAIR_IDX = {p: n for n, p in enumerate(PAIRS)}
TRIPLES = [(i, j, l) for i in range(9) for j in range(i, 9) for l in range(j, 9)]
TRI_IDX = {t: n for n, t in enumerate(TRIPLES)}


def t2_to_mono(U2):
    v = np.zeros(len(PAIRS))
    for (i, j), n in PAIR_IDX.items():
        v[n] = U2[i, j] * (1 if i == j else 2)
    return v


def t3_to_mono(U3):
    v = np.zeros(len(TRIPLES))
    for (i, j, l), n in TRI_IDX.items():
        v[n] = U3[i, j, l] * len(set(itertools.permutations((i, j, l))))
    return v


def build_functionals():
    F2, F3 = [], []
    for pi, (l1, l2, lo) in enumerate(PATHS2):
        U = path2_tensor(l1, l2, lo)
        if np.abs(U).max() < 1e-12:
            continue
        for m in range(2 * lo + 1):
            k = 0 if lo == 0 else 1 + m
            F2.append((pi, k, t2_to_mono(U[..., m])))
    for pi, (kk, l3, lo) in enumerate(PATHS3):
        U = path3_tensor(kk, l3, lo)
        if np.abs(U).max() < 1e-12:
            continue
        for m in range(2 * lo + 1):
            k = 0 if lo == 0 else 1 + m
            F3.append((pi, k, t3_to_mono(U[..., m])))
    return F2, F3


VV_ORDER = [(1, 1), (2, 2), (3, 3), (1, 2), (1, 3), (2, 3)]
WW_ORDER = ([(i, i) for i in range(4, 9)]
            + [(i, j) for i in range(4, 9) for j in range(i + 1, 9)])
VV_IDX = {p: n for n, p in enumerate(VV_ORDER)}
WW_IDX = {p: n for n, p in enumerate(WW_ORDER)}

# plane layout
#  0..8    A
#  9..14   vv block (diag 9..11 via ACT square, offdiag 12..14)
# 15..29   vw block (i-major)
# 30..44   ww block (diag 30..34 via ACT square, offdiag 35..44)
# 45..53   a0*a_j block (j=0..8)
# 54..83   vv x w          (30)
# 84..128  ww x v          (45)
# 129..146 vv x v          (18)
# 147..?   www exact
# then     a0 triples


def pair_slot_of(i, j):
    if i == 0:
        return 45 + j
    if j <= 3:
        return 9 + VV_IDX[(i, j)]
    if i >= 4:
        return 30 + WW_IDX[(i, j)]
    return 15 + (i - 1) * 5 + (j - 4)


def build_catalog():
    """Returns (prog, nplanes, tiles, coeff).
    prog: list of instr descriptors executed in order:
      ('sq', out0, in0, m)                ACT square block
      ('1d', out0, m, a0, sa, b0, sb)     out[out0+t] = buf[a0+t*sa]*buf[b0+t*sb]
      ('2d', out0, P, i0, L, i1)          out[out0+p*L+l] = buf[i0+p]*buf[i1+l]
    tiles: k-major list of (slot, k); coeff: (deg, path, k) -> [(tile, cf)].
    """
    F2, F3 = build_functionals()

    needed = sorted({TRIPLES[i] for _, _, v in F3
                     for i in np.where(np.abs(v) > 1e-12)[0]})
    www = sorted([t for t in needed if t[0] >= 4],
                 key=lambda t: (pair_slot_of(t[0], t[1]), t[2]))
    www_slot = {}
    www_prog = []
    s = 147
    i = 0
    while i < len(www):
        p0 = pair_slot_of(www[i][0], www[i][1])
        l0 = www[i][2]
        m = 1
        while (i + m < len(www)
               and pair_slot_of(www[i + m][0], www[i + m][1]) == p0
               and www[i + m][2] == l0 + m):
            m += 1
        www_prog.append(('1d', s, m, p0, 0, l0, 1))
        for t in range(m):
            www_slot[www[i + t]] = s + t
        s += m
        i += m
    a0_base = s

    def tri_slot_of(t):
        i, j, l = t
        if i == 0:
            if j == 0:
                return a0_base + 14 + l                     # (0,0,l) l<=3
            if j <= 3 and j == l:
                return a0_base + (j - 1)                    # a0*vv-diag
            if j >= 4 and j == l:
                return a0_base + 3 + (j - 4)                # a0*ww-diag
            return a0_base + 18 + (j - 1) * 5 + (l - 4)     # a0*vw
        if i >= 4:
            return www_slot[t]
        if l <= 3:
            return 129 + VV_IDX[(i, j)] * 3 + (l - 1)       # vv x v
        if j >= 4:
            return 84 + WW_IDX[(j, l)] * 3 + (i - 1)        # ww x v
        return 54 + VV_IDX[(i, j)] * 5 + (l - 4)            # vv x w

    nplanes = a0_base + 18 + 15

    # exact instrs for ww-offdiag x v (only used (pair, i) combos; slots
    # follow 84 + WW_IDX*3 + (i-1), unused slots stay garbage/unreferenced)
    wwv_off = []
    used_wwv = {}
    for (i, j, l) in needed:
        if 1 <= i <= 3 and j >= 4 and j != l:
            used_wwv.setdefault(WW_IDX[(j, l)], set()).add(i)
    for idx in sorted(used_wwv):
        vs = sorted(used_wwv[idx])
        r0 = 0
        while r0 < len(vs):
            r1 = r0 + 1
            while r1 < len(vs) and vs[r1] == vs[r1 - 1] + 1:
                r1 += 1
            wwv_off.append(('1d', 84 + idx * 3 + (vs[r0] - 1), r1 - r0,
                            30 + idx, 0, vs[r0], 1))
            r0 = r1

    # exact instrs for a0 x vw (11 of 15 used; slots a0_base+18 + vw_offset)
    a0vw = []
    offs = sorted({(j - 1) * 5 + (l - 4) for (i, j, l) in needed
                   if i == 0 and 1 <= j <= 3 and l >= 4})
    r0 = 0
    while r0 < len(offs):
        r1 = r0 + 1
        while r1 < len(offs) and offs[r1] == offs[r1 - 1] + 1:
            r1 += 1
        a0vw.append(('1d', a0_base + 18 + offs[r0], r1 - r0,
                     15 + offs[r0], 1, 0, 0))
        r0 = r1

    prog = [
        ('sq', 9, 1, 3),                 # vv diag
        ('sq', 30, 4, 5),                # ww diag
        ('1d', 12, 2, 1, 0, 2, 1),       # (1,2),(1,3)
        ('1d', 14, 1, 2, 0, 3, 1),       # (2,3)
        ('2d', 15, 3, 1, 5, 4),          # vw
        ('1d', 35, 4, 4, 0, 5, 1),       # ww offdiag
        ('1d', 39, 3, 5, 0, 6, 1),
        ('1d', 42, 2, 6, 0, 7, 1),
        ('1d', 44, 1, 7, 0, 8, 1),
        ('1d', 45, 4, 0, 0, 0, 1),       # a0 * (a0..a3); (0,j) j>=4 unused
        ('2d', 54, 6, 9, 5, 4),          # vv x w
        ('2d', 84, 5, 30, 3, 1),         # ww-diag x v (all 15 used)
        # vv x v exact (9 of 18 used): slots follow 129 + VV_IDX*3 + (l-1)
        ('1d', 129, 3, 9, 0, 1, 1),      # (1,1)x(1,2,3)
        ('1d', 133, 2, 10, 0, 2, 1),     # (2,2)x(2,3)
        ('1d', 137, 1, 11, 0, 3, 1),     # (3,3)x3
        ('1d', 139, 1, 12, 0, 2, 1),     # (1,2)x2
        ('1d', 143, 1, 13, 0, 3, 1),     # (1,3)x3
        ('1d', 146, 1, 14, 0, 3, 1),     # (2,3)x3
    ] + wwv_off + www_prog + [
        ('1d', a0_base, 3, 9, 1, 0, 0),          # a0 * vv-diag
        ('1d', a0_base + 3, 5, 30, 1, 0, 0),     # a0 * ww-diag
        ('1d', a0_base + 14, 4, 45, 1, 0, 0),    # a0 * a0a[0:4] -> (0,0,l)
    ] + a0vw

    tile_set = set()
    for pi in range(len(PATHS1)):
        lo = PATHS1[pi]
        for m in range(2 * lo + 1):
            k = 0 if lo == 0 else 1 + m
            tile_set.add((SLICES[lo][0] + m, k))
    for pi, k, v in F2:
        for i in np.where(np.abs(v) > 1e-12)[0]:
            tile_set.add((pair_slot_of(*PAIRS[i]), k))
    for pi, k, v in F3:
        for i in np.where(np.abs(v) > 1e-12)[0]:
            tile_set.add((tri_slot_of(TRIPLES[i]), k))
    tiles = sorted(tile_set)                               # slot-major
    tidx = {t: n for n, t in enumerate(tiles)}

    coeff = {}
    for pi in range(len(PATHS1)):
        lo = PATHS1[pi]
        for m in range(2 * lo + 1):
            k = 0 if lo == 0 else 1 + m
            coeff.setdefault((1, pi, k), []).append(
                (tidx[(SLICES[lo][0] + m, k)], 1.0))
    for pi, k, v in F2:
        for i in np.where(np.abs(v) > 1e-12)[0]:
            coeff.setdefault((2, pi, k), []).append(
                (tidx[(pair_slot_of(*PAIRS[i]), k)], v[i]))
    for pi, k, v in F3:
        for i in np.where(np.abs(v) > 1e-12)[0]:
            coeff.setdefault((3, pi, k), []).append(
                (tidx[(tri_slot_of(TRIPLES[i]), k)], v[i]))
    return prog, nplanes, tiles, coeff


def build_lambda(tiles, coeff, w1, w2, w3):
    """lam[s, c, tile_index]"""
    wd = {1: w1, 2: w2, 3: w3}
    lam = np.zeros((N_ELEM, CHANNELS, len(tiles)))
    for (d, pi, k), lst in coeff.items():
        w = wd[d][:, pi, :]
        for ti, cf in lst:
            lam[:, :, ti] += w * cf
    return lam


_CATALOG = None


def catalog():
    global _CATALOG
    if _CATALOG is None:
        _CATALOG = build_catalog()
    return _CATALOG


# --------------------------------------------------------------------------
# Bass program
# --------------------------------------------------------------------------

_COMPILED = None


def build_bass(prog, nplanes, tiles):
    from concourse import bacc
    from concourse import tile
    from concourse import mybir

    NT = len(tiles)
    NCH = (NT + PSI_CHT - 1) // PSI_CHT
    bf = mybir.dt.bfloat16
    f32 = mybir.dt.float32
    nc = bacc.Bacc(None, target_bir_lowering=False)

    feats_d = nc.dram_tensor("feats", [WPC, 128, 9, WIN], bf, kind="ExternalInput")
    psi_d = nc.dram_tensor("psi", [WPC, NCH, 128, PSI_CHT, 128], bf,
                           kind="ExternalInput")
    out_d = nc.dram_tensor("out", [WPC, 128, 4, WIN], f32, kind="ExternalOutput")

    # start/stop bookkeeping per k
    first_t = {}
    last_t = {}
    for t, (slot, k) in enumerate(tiles):
        first_t.setdefault(k, t)
        last_t[k] = t

    with tile.TileContext(nc) as tc:
        with (
            tc.tile_pool(name="buf", bufs=2) as bufp,
            tc.tile_pool(name="psir", bufs=4) as psir,
            tc.tile_pool(name="outp", bufs=2) as outp,
            tc.tile_pool(name="psum", bufs=2, space="PSUM") as psump,
        ):
            for w in range(WPC):
                planes = bufp.tile([128, nplanes, WIN], bf, tag="planes")
                nc.sync.dma_start(planes[:, 0:9, :], feats_d[w])

                for ins in prog:
                    if ins[0] == 'sq':
                        _, o, a0, m = ins
                        nc.scalar.activation(
                            planes[:, o:o + m, :], planes[:, a0:a0 + m, :],
                            mybir.ActivationFunctionType.Square)
                    elif ins[0] == '1d':
                        _, o, m, a0, sa, b0, sb = ins
                        if sa == 1:
                            in0 = planes[:, a0:a0 + m, :]
                        else:
                            in0 = planes[:, a0:a0 + 1, :].broadcast_to(
                                [128, m, WIN])
                        if sb == 1:
                            in1 = planes[:, b0:b0 + m, :]
                        else:
                            in1 = planes[:, b0:b0 + 1, :].broadcast_to(
                                [128, m, WIN])
                        nc.vector.tensor_tensor(
                            out=planes[:, o:o + m, :], in0=in0, in1=in1,
                            op=mybir.AluOpType.mult)
                    else:
                        _, o, P, i0, L, i1 = ins
                        out_ap = planes[:, o:o + P * L, :].rearrange(
                            "c (p l) n -> c p l n", p=P)
                        in0 = planes[:, i0:i0 + P, :].unsqueeze(2).broadcast_to(
                            [128, P, L, WIN])
                        in1 = planes[:, i1:i1 + L, :].unsqueeze(1).broadcast_to(
                            [128, P, L, WIN])
                        nc.vector.tensor_tensor(
                            out=out_ap, in0=in0, in1=in1,
                            op=mybir.AluOpType.mult)

                out_ps0 = psump.tile([128, WIN], f32, tag="ops0")
                out_ps1 = psump.tile([128, WIN], f32, tag="ops1")
                out_ps2 = psump.tile([128, WIN], f32, tag="ops2")
                out_ps3 = psump.tile([128, WIN], f32, tag="ops3")
                out_ps = [out_ps0, out_ps1, out_ps2, out_ps3]
                psi_t = None
                for t, (slot, k) in enumerate(tiles):
                    ch, off = divmod(t, PSI_CHT)
                    if off == 0:
                        psi_t = psir.tile([128, PSI_CHT, 128], bf, tag="psi")
                        nc.sync.dma_start(psi_t[:], psi_d[w, ch])
                    nc.tensor.matmul(
                        out_ps[k][:], psi_t[:, off, :], planes[:, slot, :],
                        start=(t == first_t[k]), stop=(t == last_t[k]))

                out_t = outp.tile([128, 4, WIN], f32, tag="out")
                for k in range(4):
                    nc.scalar.activation(
                        out_t[:, k, :], out_ps[k][:],
                        mybir.ActivationFunctionType.Copy)
                nc.sync.dma_start(out_d[w], out_t[:])

    nc.compile()
    return nc


# --------------------------------------------------------------------------
# Host prep + run
# --------------------------------------------------------------------------

def _plan_windows(species):
    order = np.argsort(species, kind='stable')
    win_nodes = np.zeros((NWIN, WIN), np.int64)
    win_spec = np.zeros(NWIN, np.int64)
    win_valid = np.zeros((NWIN, WIN), bool)
    w = 0
    for s in range(N_ELEM):
        idx = order[species[order] == s]
        for c0 in range(0, len(idx), WIN):
            chunk = idx[c0:c0 + WIN]
            n = len(chunk)
            assert w < NWIN, "window overflow"
            win_nodes[w, :n] = chunk
            win_nodes[w, n:] = chunk[0] if n else 0
            win_valid[w, :n] = True
            win_spec[w] = s
            w += 1
    return win_nodes, win_spec, win_valid


_LAST_IN_MAPS = None


def kernel(node_feats, w1, w2, w3, lin_w0, lin_w1, species):
    global _COMPILED, _LAST_IN_MAPS
    import ml_dtypes
    from concourse.bass_utils import run_bass_kernel_spmd

    node_feats = np.ascontiguousarray(np.asarray(node_feats, np.float32))
    species = np.asarray(species)
    prog, nplanes, tiles, coeff = catalog()
    lam = build_lambda(tiles, coeff,
                       np.asarray(w1, np.float64), np.asarray(w2, np.float64),
                       np.asarray(w3, np.float64))       # [S, C, NT] f64
    NT = len(tiles)
    NCH = (NT + PSI_CHT - 1) // PSI_CHT

    if _COMPILED is None:
        _COMPILED = build_bass(prog, nplanes, tiles)
    nc = _COMPILED

    win_nodes, win_spec, win_valid = _plan_windows(species)

    # psi[s, t, c, f] = lam[s, c, t] * lin_{k(t)}[c, f] / sqrt(C)
    sc = 1.0 / math.sqrt(CHANNELS)
    lin = np.stack([np.asarray(lin_w0, np.float64),
                    np.asarray(lin_w1, np.float64)], axis=0) * sc  # [2, C, F]
    ks = np.array([0 if k == 0 else 1 for (slot, k) in tiles])
    lin_per_tile = lin[ks]                                  # [NT, C, F]
    psi_s = np.einsum('sct,tcf->stcf', lam, lin_per_tile)   # [S, NT, C, F]
    psi_s = psi_s.astype(ml_dtypes.bfloat16)

    # per window, chunked layout [NWIN, NCH, C, CHT, F]
    psi_win = np.zeros((NWIN, NCH, CHANNELS, PSI_CHT, N_FEAT), ml_dtypes.bfloat16)
    dummy = ~win_valid.any(axis=1)
    for w in range(NWIN):
        if dummy[w]:
            continue
        p = psi_s[win_spec[w]]                              # [NT, C, F]
        for ch in range(NCH):
            t0 = ch * PSI_CHT
            t1 = min(t0 + PSI_CHT, NT)
            psi_win[w, ch, :, :t1 - t0, :] = p[t0:t1].transpose(1, 0, 2)

    fw = node_feats[win_nodes]                   # [NWIN, WIN, C, 9]
    feats_win = np.ascontiguousarray(
        fw.transpose(0, 2, 3, 1)).astype(ml_dtypes.bfloat16)

    in_maps = []
    for core in range(8):
        ws = slice(core * WPC, (core + 1) * WPC)
        in_maps.append({
            "feats": feats_win[ws],
            "psi": psi_win[ws],
        })
    _LAST_IN_MAPS = in_maps
    res = run_bass_kernel_spmd(nc, in_maps, list(range(8)))

    out = np.zeros((N_NODES, N_FEAT, 4), np.float32)
    for core in range(8):
        o = np.asarray(res.results[core]["out"], np.float32)
        for wi in range(WPC):
            w = core * WPC + wi
            v = win_valid[w]
            if v.any():
                out[win_nodes[w, v]] = o[wi].transpose(2, 0, 1)[v]
    return out


# revision 35
# speedup vs baseline: 1.0861x; 1.0036x over previous
"""Trainium2 Bass kernel for nn_EquivariantProductBasisBlock.

Architecture (v3):
- Host: sort nodes by species into species-pure windows of WIN nodes
  (NWIN total, WPC per core; data-parallel over nodes on 8 cores).
  For every reduction tile (plane q -> output comp k) the host ships a
  pre-fused stationary matrix  Psi[t][c,f] = lambda_qk[species,c] *
  lin_k[c,f] / sqrt(C)  (bf16), where lambda collapses the CG-structure
  path weights (w1/w2/w3).
- Device, layout [channel=128 partitions, nodes free]:
  * ACT:    squares of the 9 irrep components
  * DVE+GPSIMD: remaining pair products and triple products (batched
    strided tensor_tensor, bf16)
  * PE:     out_k[f,n] += Psi[t]^T @ plane_q[c,n] accumulated in PSUM
            (the species weighting AND channel->feature Linear both live
            in the stationary; one matmul per tile)
- Host: gather windows back to the original node order.
"""

import math
import itertools
import sys

import numpy as np

sys.path.insert(0, "/opt/trn_rl_repo")

N_NODES, CHANNELS, N_ELEM, N_FEAT = 4096, 128, 10, 128
LS_IN = [0, 1, 2]
L_OUT = [0, 1]
L12_MAX = 3
SLICES = {0: (0, 1), 1: (1, 4), 2: (4, 9)}
PATHS1 = [l for l in L_OUT if l in LS_IN]
PATHS2 = [(l1, l2, lo) for l1 in LS_IN for l2 in LS_IN for lo in L_OUT
          if abs(l1 - l2) <= lo <= l1 + l2]
T12_KEYS = [(l1, l2, l12) for l1 in LS_IN for l2 in LS_IN
            for l12 in range(abs(l1 - l2), min(l1 + l2, L12_MAX) + 1)]
PATHS3 = [(k, l3, lo) for k in T12_KEYS for l3 in LS_IN for lo in L_OUT
          if abs(k[2] - l3) <= lo <= k[2] + l3]

WIN = 208          # nodes per window
NWIN = 24          # total windows
WPC = NWIN // 8    # windows per core
PSI_CHT = 33       # psi tiles per DMA chunk


# --------------------------------------------------------------------------
# CG / path-tensor algebra (host-side, numpy)
# --------------------------------------------------------------------------

def _su2_cg(j1, m1, j2, m2, j3, m3):
    if m3 != m1 + m2:
        return 0.0
    f = math.factorial
    vmin = max(-j1 + j2 + m3, -j1 + m1, 0)
    vmax = min(j2 + j3 + m1, j3 - j1 + j2, j3 + m3)
    C = math.sqrt((2 * j3 + 1) * f(j3 + j1 - j2) * f(j3 - j1 + j2)
                  * f(j1 + j2 - j3) * f(j3 + m3) * f(j3 - m3)
                  / (f(j1 + j2 + j3 + 1) * f(j1 - m1) * f(j1 + m1)
                     * f(j2 - m2) * f(j2 + m2)))
    S = 0.0
    for v in range(vmin, vmax + 1):
        S += (-1) ** (v + j2 + m2) * f(j2 + j3 + m1 - v) * f(j1 - m1 + v) / (
            f(v) * f(j3 - j1 + j2 - v) * f(j3 + m3 - v) * f(v + j1 - j2 - m3))
    return C * S


def _c2r(l):
    q = np.zeros((2 * l + 1, 2 * l + 1), dtype=np.complex128)
    for m in range(-l, 0):
        q[l + m, l + abs(m)] = 1.0 / math.sqrt(2)
        q[l + m, l - abs(m)] = -1j / math.sqrt(2)
    q[l, l] = 1.0
    for m in range(1, l + 1):
        q[l + m, l + abs(m)] = (-1) ** m / math.sqrt(2)
        q[l + m, l - abs(m)] = 1j * (-1) ** m / math.sqrt(2)
    return (-1j) ** l * q


_CG_CACHE = {}


def real_cg(l1, l2, l3):
    key = (l1, l2, l3)
    if key not in _CG_CACHE:
        Cc = np.zeros((2 * l1 + 1, 2 * l2 + 1, 2 * l3 + 1), dtype=np.complex128)
        for i, m1 in enumerate(range(-l1, l1 + 1)):
            for j, m2 in enumerate(range(-l2, l2 + 1)):
                for k, m3 in enumerate(range(-l3, l3 + 1)):
                    Cc[i, j, k] = _su2_cg(l1, m1, l2, m2, l3, m3)
        R = np.einsum('ij,kl,mn,ikn->jlm', _c2r(l1), _c2r(l2),
                      np.conj(_c2r(l3).T), Cc)
        _CG_CACHE[key] = np.real(R)
    return _CG_CACHE[key]


def path2_tensor(l1, l2, lo):
    cg = real_cg(l1, l2, lo)
    U = np.zeros((9, 9, 2 * lo + 1))
    s1, e1 = SLICES[l1]
    s2, e2 = SLICES[l2]
    U[s1:e1, s2:e2, :] = cg
    return 0.5 * (U + U.transpose(1, 0, 2))


def path3_tensor(kk, l3, lo):
    l1, l2, l12 = kk
    T = np.einsum('abm,mcn->abcn', real_cg(l1, l2, l12), real_cg(l12, l3, lo))
    U = np.zeros((9, 9, 9, 2 * lo + 1))
    s1, e1 = SLICES[l1]
    s2, e2 = SLICES[l2]
    s3, e3 = SLICES[l3]
    U[s1:e1, s2:e2, s3:e3, :] = T
    S = np.zeros_like(U)
    for perm in itertools.permutations([0, 1, 2]):
        S += U.transpose(*perm, 3)
    return S / 6.0


PAIRS = [(i, j) for i in range(9) for j in range(i, 9)]
PAIR_IDX = {p: n for n, p in enumerate(PAIRS)}
TRIPLES = [(i, j, l) for i in range(9) for j in range(i, 9) for l in range(j, 9)]
TRI_IDX = {t: n for n, t in enumerate(TRIPLES)}


def t2_to_mono(U2):
    v = np.zeros(len(PAIRS))
    for (i, j), n in PAIR_IDX.items():
        v[n] = U2[i, j] * (1 if i == j else 2)
    return v


def t3_to_mono(U3):
    v = np.zeros(len(TRIPLES))
    for (i, j, l), n in TRI_IDX.items():
        v[n] = U3[i, j, l] * len(set(itertools.permutations((i, j, l))))
    return v


def build_functionals():
    F2, F3 = [], []
    for pi, (l1, l2, lo) in enumerate(PATHS2):
        U = path2_tensor(l1, l2, lo)
        if np.abs(U).max() < 1e-12:
            continue
        for m in range(2 * lo + 1):
            k = 0 if lo == 0 else 1 + m
            F2.append((pi, k, t2_to_mono(U[..., m])))
    for pi, (kk, l3, lo) in enumerate(PATHS3):
        U = path3_tensor(kk, l3, lo)
        if np.abs(U).max() < 1e-12:
            continue
        for m in range(2 * lo + 1):
            k = 0 if lo == 0 else 1 + m
            F3.append((pi, k, t3_to_mono(U[..., m])))
    return F2, F3


VV_ORDER = [(1, 1), (2, 2), (3, 3), (1, 2), (1, 3), (2, 3)]
WW_ORDER = ([(i, i) for i in range(4, 9)]
            + [(i, j) for i in range(4, 9) for j in range(i + 1, 9)])
VV_IDX = {p: n for n, p in enumerate(VV_ORDER)}
WW_IDX = {p: n for n, p in enumerate(WW_ORDER)}

# plane layout
#  0..8    A
#  9..14   vv block (diag 9..11 via ACT square, offdiag 12..14)
# 15..29   vw block (i-major)
# 30..44   ww block (diag 30..34 via ACT square, offdiag 35..44)
# 45..53   a0*a_j block (j=0..8)
# 54..83   vv x w          (30)
# 84..128  ww x v          (45)
# 129..146 vv x v          (18)
# 147..?   www exact
# then     a0 triples


def pair_slot_of(i, j):
    if i == 0:
        return 45 + j
    if j <= 3:
        return 9 + VV_IDX[(i, j)]
    if i >= 4:
        return 30 + WW_IDX[(i, j)]
    return 15 + (i - 1) * 5 + (j - 4)


def build_catalog():
    """Returns (prog, nplanes, tiles, coeff).
    prog: list of instr descriptors executed in order:
      ('sq', out0, in0, m)                ACT square block
      ('1d', out0, m, a0, sa, b0, sb)     out[out0+t] = buf[a0+t*sa]*buf[b0+t*sb]
      ('2d', out0, P, i0, L, i1)          out[out0+p*L+l] = buf[i0+p]*buf[i1+l]
    tiles: k-major list of (slot, k); coeff: (deg, path, k) -> [(tile, cf)].
    """
    F2, F3 = build_functionals()

    needed = sorted({TRIPLES[i] for _, _, v in F3
                     for i in np.where(np.abs(v) > 1e-12)[0]})
    www = sorted([t for t in needed if t[0] >= 4],
                 key=lambda t: (pair_slot_of(t[0], t[1]), t[2]))
    www_slot = {}
    www_prog = []
    s = 147
    i = 0
    while i < len(www):
        p0 = pair_slot_of(www[i][0], www[i][1])
        l0 = www[i][2]
        m = 1
        while (i + m < len(www)
               and pair_slot_of(www[i + m][0], www[i + m][1]) == p0
               and www[i + m][2] == l0 + m):
            m += 1
        www_prog.append(('1d', s, m, p0, 0, l0, 1))
        for t in range(m):
            www_slot[www[i + t]] = s + t
        s += m
        i += m
    a0_base = s

    def tri_slot_of(t):
        i, j, l = t
        if i == 0:
            if j == 0:
                return a0_base + 14 + l                     # (0,0,l) l<=3
            if j <= 3 and j == l:
                return a0_base + (j - 1)                    # a0*vv-diag
            if j >= 4 and j == l:
                return a0_base + 3 + (j - 4)                # a0*ww-diag
            return a0_base + 18 + (j - 1) * 5 + (l - 4)     # a0*vw
        if i >= 4:
            return www_slot[t]
        if l <= 3:
            return 129 + VV_IDX[(i, j)] * 3 + (l - 1)       # vv x v
        if j >= 4:
            return 84 + WW_IDX[(j, l)] * 3 + (i - 1)        # ww x v
        return 54 + VV_IDX[(i, j)] * 5 + (l - 4)            # vv x w

    nplanes = a0_base + 18 + 15

    # exact instrs for ww-offdiag x v (only used (pair, i) combos; slots
    # follow 84 + WW_IDX*3 + (i-1), unused slots stay garbage/unreferenced)
    wwv_off = []
    used_wwv = {}
    for (i, j, l) in needed:
        if 1 <= i <= 3 and j >= 4 and j != l:
            used_wwv.setdefault(WW_IDX[(j, l)], set()).add(i)
    for idx in sorted(used_wwv):
        vs = sorted(used_wwv[idx])
        r0 = 0
        while r0 < len(vs):
            r1 = r0 + 1
            while r1 < len(vs) and vs[r1] == vs[r1 - 1] + 1:
                r1 += 1
            wwv_off.append(('1d', 84 + idx * 3 + (vs[r0] - 1), r1 - r0,
                            30 + idx, 0, vs[r0], 1))
            r0 = r1

    # exact instrs for vv x w (27 of 30 used; slots 54 + VV_IDX*5 + (l-4))
    vvw_ex = []
    used_vvw = {}
    for (i, j, l) in needed:
        if 1 <= i and j <= 3 and l >= 4:
            used_vvw.setdefault(VV_IDX[(i, j)], set()).add(l)
    for idx in sorted(used_vvw):
        ls = sorted(used_vvw[idx])
        r0 = 0
        while r0 < len(ls):
            r1 = r0 + 1
            while r1 < len(ls) and ls[r1] == ls[r1 - 1] + 1:
                r1 += 1
            vvw_ex.append(('1d', 54 + idx * 5 + (ls[r0] - 4), r1 - r0,
                           9 + idx, 0, ls[r0], 1))
            r0 = r1

    # exact instrs for a0 x vw (11 of 15 used; slots a0_base+18 + vw_offset)
    a0vw = []
    offs = sorted({(j - 1) * 5 + (l - 4) for (i, j, l) in needed
                   if i == 0 and 1 <= j <= 3 and l >= 4})
    r0 = 0
    while r0 < len(offs):
        r1 = r0 + 1
        while r1 < len(offs) and offs[r1] == offs[r1 - 1] + 1:
            r1 += 1
        a0vw.append(('1d', a0_base + 18 + offs[r0], r1 - r0,
                     15 + offs[r0], 1, 0, 0))
        r0 = r1

    prog = [
        ('sq', 9, 1, 3),                 # vv diag
        ('sq', 30, 4, 5),                # ww diag
        ('1d', 12, 2, 1, 0, 2, 1),       # (1,2),(1,3)
        ('1d', 14, 1, 2, 0, 3, 1),       # (2,3)
        ('2d', 15, 3, 1, 5, 4),          # vw
        ('1d', 35, 3, 4, 0, 5, 1),       # ww offdiag ((4,8) unused)
        ('1d', 39, 3, 5, 0, 6, 1),
        ('1d', 42, 2, 6, 0, 7, 1),
        ('1d', 44, 1, 7, 0, 8, 1),
        ('1d', 45, 4, 0, 0, 0, 1),       # a0 * (a0..a3); (0,j) j>=4 unused
    ] + vvw_ex + [
        ('2d', 84, 5, 30, 3, 1),         # ww-diag x v (all 15 used)
        # vv x v exact (9 of 18 used): slots follow 129 + VV_IDX*3 + (l-1)
        ('1d', 129, 3, 9, 0, 1, 1),      # (1,1)x(1,2,3)
        ('1d', 133, 2, 10, 0, 2, 1),     # (2,2)x(2,3)
        ('1d', 137, 1, 11, 0, 3, 1),     # (3,3)x3
        ('1d', 139, 1, 12, 0, 2, 1),     # (1,2)x2
        ('1d', 143, 1, 13, 0, 3, 1),     # (1,3)x3
        ('1d', 146, 1, 14, 0, 3, 1),     # (2,3)x3
    ] + wwv_off + www_prog + [
        ('1d', a0_base, 3, 9, 1, 0, 0),          # a0 * vv-diag
        ('1d', a0_base + 3, 5, 30, 1, 0, 0),     # a0 * ww-diag
        ('1d', a0_base + 14, 4, 45, 1, 0, 0),    # a0 * a0a[0:4] -> (0,0,l)
    ] + a0vw

    tile_set = set()
    for pi in range(len(PATHS1)):
        lo = PATHS1[pi]
        for m in range(2 * lo + 1):
            k = 0 if lo == 0 else 1 + m
            tile_set.add((SLICES[lo][0] + m, k))
    for pi, k, v in F2:
        for i in np.where(np.abs(v) > 1e-12)[0]:
            tile_set.add((pair_slot_of(*PAIRS[i]), k))
    for pi, k, v in F3:
        for i in np.where(np.abs(v) > 1e-12)[0]:
            tile_set.add((tri_slot_of(TRIPLES[i]), k))
    tiles = sorted(tile_set)                               # slot-major
    tidx = {t: n for n, t in enumerate(tiles)}

    coeff = {}
    for pi in range(len(PATHS1)):
        lo = PATHS1[pi]
        for m in range(2 * lo + 1):
            k = 0 if lo == 0 else 1 + m
            coeff.setdefault((1, pi, k), []).append(
                (tidx[(SLICES[lo][0] + m, k)], 1.0))
    for pi, k, v in F2:
        for i in np.where(np.abs(v) > 1e-12)[0]:
            coeff.setdefault((2, pi, k), []).append(
                (tidx[(pair_slot_of(*PAIRS[i]), k)], v[i]))
    for pi, k, v in F3:
        for i in np.where(np.abs(v) > 1e-12)[0]:
            coeff.setdefault((3, pi, k), []).append(
                (tidx[(tri_slot_of(TRIPLES[i]), k)], v[i]))
    return prog, nplanes, tiles, coeff


def build_lambda(tiles, coeff, w1, w2, w3):
    """lam[s, c, tile_index]"""
    wd = {1: w1, 2: w2, 3: w3}
    lam = np.zeros((N_ELEM, CHANNELS, len(tiles)))
    for (d, pi, k), lst in coeff.items():
        w = wd[d][:, pi, :]
        for ti, cf in lst:
            lam[:, :, ti] += w * cf
    return lam


_CATALOG = None


def catalog():
    global _CATALOG
    if _CATALOG is None:
        _CATALOG = build_catalog()
    return _CATALOG


# --------------------------------------------------------------------------
# Bass program
# --------------------------------------------------------------------------

_COMPILED = None


def build_bass(prog, nplanes, tiles):
    from concourse import bacc
    from concourse import tile
    from concourse import mybir

    NT = len(tiles)
    NCH = (NT + PSI_CHT - 1) // PSI_CHT
    bf = mybir.dt.bfloat16
    f32 = mybir.dt.float32
    nc = bacc.Bacc(None, target_bir_lowering=False)

    feats_d = nc.dram_tensor("feats", [WPC, 128, 9, WIN], bf, kind="ExternalInput")
    psi_d = nc.dram_tensor("psi", [WPC, NCH, 128, PSI_CHT, 128], bf,
                           kind="ExternalInput")
    out_d = nc.dram_tensor("out", [WPC, 128, 4, WIN], f32, kind="ExternalOutput")

    # start/stop bookkeeping per k
    first_t = {}
    last_t = {}
    for t, (slot, k) in enumerate(tiles):
        first_t.setdefault(k, t)
        last_t[k] = t

    with tile.TileContext(nc) as tc:
        with (
            tc.tile_pool(name="buf", bufs=2) as bufp,
            tc.tile_pool(name="psir", bufs=4) as psir,
            tc.tile_pool(name="outp", bufs=2) as outp,
            tc.tile_pool(name="psum", bufs=2, space="PSUM") as psump,
        ):
            for w in range(WPC):
                planes = bufp.tile([128, nplanes, WIN], bf, tag="planes")
                nc.sync.dma_start(planes[:, 0:9, :], feats_d[w])

                for ins in prog:
                    if ins[0] == 'sq':
                        _, o, a0, m = ins
                        nc.scalar.activation(
                            planes[:, o:o + m, :], planes[:, a0:a0 + m, :],
                            mybir.ActivationFunctionType.Square)
                    elif ins[0] == '1d':
                        _, o, m, a0, sa, b0, sb = ins
                        if sa == 1:
                            in0 = planes[:, a0:a0 + m, :]
                        else:
                            in0 = planes[:, a0:a0 + 1, :].broadcast_to(
                                [128, m, WIN])
                        if sb == 1:
                            in1 = planes[:, b0:b0 + m, :]
                        else:
                            in1 = planes[:, b0:b0 + 1, :].broadcast_to(
                                [128, m, WIN])
                        nc.vector.tensor_tensor(
                            out=planes[:, o:o + m, :], in0=in0, in1=in1,
                            op=mybir.AluOpType.mult)
                    else:
                        _, o, P, i0, L, i1 = ins
                        out_ap = planes[:, o:o + P * L, :].rearrange(
                            "c (p l) n -> c p l n", p=P)
                        in0 = planes[:, i0:i0 + P, :].unsqueeze(2).broadcast_to(
                            [128, P, L, WIN])
                        in1 = planes[:, i1:i1 + L, :].unsqueeze(1).broadcast_to(
                            [128, P, L, WIN])
                        nc.vector.tensor_tensor(
                            out=out_ap, in0=in0, in1=in1,
                            op=mybir.AluOpType.mult)

                out_ps0 = psump.tile([128, WIN], f32, tag="ops0")
                out_ps1 = psump.tile([128, WIN], f32, tag="ops1")
                out_ps2 = psump.tile([128, WIN], f32, tag="ops2")
                out_ps3 = psump.tile([128, WIN], f32, tag="ops3")
                out_ps = [out_ps0, out_ps1, out_ps2, out_ps3]
                psi_t = None
                for t, (slot, k) in enumerate(tiles):
                    ch, off = divmod(t, PSI_CHT)
                    if off == 0:
                        psi_t = psir.tile([128, PSI_CHT, 128], bf, tag="psi")
                        nc.sync.dma_start(psi_t[:], psi_d[w, ch])
                    nc.tensor.matmul(
                        out_ps[k][:], psi_t[:, off, :], planes[:, slot, :],
                        start=(t == first_t[k]), stop=(t == last_t[k]))

                out_t = outp.tile([128, 4, WIN], f32, tag="out")
                for k in range(4):
                    nc.scalar.activation(
                        out_t[:, k, :], out_ps[k][:],
                        mybir.ActivationFunctionType.Copy)
                nc.sync.dma_start(out_d[w], out_t[:])

    nc.compile()
    return nc


# --------------------------------------------------------------------------
# Host prep + run
# --------------------------------------------------------------------------

def _plan_windows(species):
    order = np.argsort(species, kind='stable')
    win_nodes = np.zeros((NWIN, WIN), np.int64)
    win_spec = np.zeros(NWIN, np.int64)
    win_valid = np.zeros((NWIN, WIN), bool)
    w = 0
    for s in range(N_ELEM):
        idx = order[species[order] == s]
        for c0 in range(0, len(idx), WIN):
            chunk = idx[c0:c0 + WIN]
            n = len(chunk)
            assert w < NWIN, "window overflow"
            win_nodes[w, :n] = chunk
            win_nodes[w, n:] = chunk[0] if n else 0
            win_valid[w, :n] = True
            win_spec[w] = s
            w += 1
    return win_nodes, win_spec, win_valid


_LAST_IN_MAPS = None


def kernel(node_feats, w1, w2, w3, lin_w0, lin_w1, species):
    global _COMPILED, _LAST_IN_MAPS
    import ml_dtypes
    from concourse.bass_utils import run_bass_kernel_spmd

    node_feats = np.ascontiguousarray(np.asarray(node_feats, np.float32))
    species = np.asarray(species)
    prog, nplanes, tiles, coeff = catalog()
    lam = build_lambda(tiles, coeff,
                       np.asarray(w1, np.float64), np.asarray(w2, np.float64),
                       np.asarray(w3, np.float64))       # [S, C, NT] f64
    NT = len(tiles)
    NCH = (NT + PSI_CHT - 1) // PSI_CHT

    if _COMPILED is None:
        _COMPILED = build_bass(prog, nplanes, tiles)
    nc = _COMPILED

    win_nodes, win_spec, win_valid = _plan_windows(species)

    # psi[s, t, c, f] = lam[s, c, t] * lin_{k(t)}[c, f] / sqrt(C)
    sc = 1.0 / math.sqrt(CHANNELS)
    lin = np.stack([np.asarray(lin_w0, np.float64),
                    np.asarray(lin_w1, np.float64)], axis=0) * sc  # [2, C, F]
    ks = np.array([0 if k == 0 else 1 for (slot, k) in tiles])
    lin_per_tile = lin[ks]                                  # [NT, C, F]
    psi_s = np.einsum('sct,tcf->stcf', lam, lin_per_tile)   # [S, NT, C, F]
    psi_s = psi_s.astype(ml_dtypes.bfloat16)

    # per window, chunked layout [NWIN, NCH, C, CHT, F]
    psi_win = np.zeros((NWIN, NCH, CHANNELS, PSI_CHT, N_FEAT), ml_dtypes.bfloat16)
    dummy = ~win_valid.any(axis=1)
    for w in range(NWIN):
        if dummy[w]:
            continue
        p = psi_s[win_spec[w]]                              # [NT, C, F]
        for ch in range(NCH):
            t0 = ch * PSI_CHT
            t1 = min(t0 + PSI_CHT, NT)
            psi_win[w, ch, :, :t1 - t0, :] = p[t0:t1].transpose(1, 0, 2)

    fw = node_feats[win_nodes]                   # [NWIN, WIN, C, 9]
    feats_win = np.ascontiguousarray(
        fw.transpose(0, 2, 3, 1)).astype(ml_dtypes.bfloat16)

    in_maps = []
    for core in range(8):
        ws = slice(core * WPC, (core + 1) * WPC)
        in_maps.append({
            "feats": feats_win[ws],
            "psi": psi_win[ws],
        })
    _LAST_IN_MAPS = in_maps
    res = run_bass_kernel_spmd(nc, in_maps, list(range(8)))

    out = np.zeros((N_NODES, N_FEAT, 4), np.float32)
    for core in range(8):
        o = np.asarray(res.results[core]["out"], np.float32)
        for wi in range(WPC):
            w = core * WPC + wi
            v = win_valid[w]
            if v.any():
                out[win_nodes[w, v]] = o[wi].transpose(2, 0, 1)[v]
    return out
